# revision 33
# baseline (speedup 1.0000x reference)
"""Attention-GRU decoder (teacher forcing) on 8 TRN2 NeuronCores.

Strategy (v3):
  Phase 0 (per core, batch sharded 4 seqs/core): precompute
     EcT    = (enc @ W1_enc.T).T + b1   -- attention enc projection
     EncWc  = enc @ Wc.T                -- context->GRU-input projection
     GIX    = x_aug @ Wx_aug            -- all-steps input projection + biases
   Inputs stream on 5 parallel DMA rings (tensor ring carries the
   step-1-critical recurrence weights; W_out is emitted last on the
   vector ring so it cannot starve the phase-0 streams).
  Phase 1: 31 sequential steps, data-parallel over batch (BC=4/core).
     h @ W1h.T and h @ W_hh.T are fp8 stationary matmuls (LDW-bound,
     N=4 moving).  Attention softmax uses direct Exp (exp+tanh share the
     exp_and_others ACT table set, so no table reloads); gate sigmoids
     stay in tanh form.  e-score matmuls are emitted before the gh block
     so the scheduler runs them as soon as the aw tanh halves land,
     hiding the softmax chain under the gh matmul stream.  h is written
     fp16-first so next-step matmuls start immediately.  Partial fp16
     AllGathers of h every 4 steps (staged per-step) gather straight
     into the (t, rank, b) hgat buffer via strided DMA; vocab-projection
     matmuls fill the PE idle windows, their outputs (+b_out) are
     written directly to the output tensor and exp-summed on the fly.
  Tail: last AllGather + leftover vocab matmuls + exp accumulation,
     one [P, NCH] expsum output.  The final log-softmax shift
     (out -= log(sum_cores expsum)) is folded into the host-side
     unshard (all O(V) reduction work stays on-chip).

kernel(**inputs) takes full inputs, returns [B, T-1, V] float32.
"""
import numpy as np

import concourse.bacc as bacc
import concourse.bass as bass
import concourse.mybir as mybir
import concourse.tile as tile
from concourse.bass_utils import run_bass_kernel_spmd

F32 = mybir.dt.float32
F16 = mybir.dt.float16
F8 = mybir.dt.float8e4
AF = mybir.ActivationFunctionType
ALU = mybir.AluOpType
WS = 64.0            # fp8 weight scale
WSI = 1.0 / WS

B, S, H, V, Dw, T = 32, 50, 1024, 32000, 512, 32
NCORES = 8
P = 128
TS = T - 1            # 31 decode steps
BC = B // NCORES      # 4 sequences per core
VC = V // NCORES      # 4000 vocab rows per core
SP = 64               # padded s-block per sequence
NBS = BC * SP         # 256 padded (b,s) columns per core
KH = H // P           # 8 hidden chunks
KG = 3 * H // P       # 24 gate chunks
NV = 8                # vocab n-chunks per core
NVS = VC // NV        # 500
TCH = 4               # steps per AllGather chunk
NCH = 8               # number of chunks (last has 3 steps)
NWC = 12              # EncWc column chunks (256 wide)

_CACHE = {}


def _chunk(j):
    tlo = TCH * j + 1
    thi = min(tlo + TCH, T)
    return tlo, thi


def _build():
    nc = bacc.Bacc("TRN2", target_bir_lowering=False, debug=False,
                   num_devices=NCORES)

    def din(name, shape, dt):
        return nc.dram_tensor(name, shape, dt, kind="ExternalInput").ap()

    enct16_d = din("enct16", [P, KH, NBS], F16)
    w1et16_d = din("w1et16", [P, KH, H], F8)
    wct16_d = din("wct16", [P, KH, 3 * H], F8)
    wxat8_d = din("wxat8", [P, 4, 3 * H], F8)
    wxb16_d = din("wxb16", [P, 3 * H], F16)
    xat16_d = din("xat16", [P, 5, P], F16)
    whht_d = din("whht", [P, KH, 3 * H], F8)
    w1ht_d = din("w1ht", [P, KH, H], F8)
    w2t16_d = din("w2t16", [P, KH], F16)
    b1t_d = din("b1t", [P, KH], F32)
    bhnrep_d = din("bhnrep", [P, KH * BC], F32)
    h0t_d = din("h0t", [P, KH * BC], F32)
    woutt16_d = din("woutt16", [P, KH, VC], F16)
    bout16_d = din("bout16", [1, VC], F16)
    out_d = nc.dram_tensor("out", [NCH, P, VC], F16,
                           kind="ExternalOutput").ap()
    sums_d = nc.dram_tensor("sums", [P, NCH], F32,
                            kind="ExternalOutput").ap()

    rg = [list(range(NCORES))]

    with tile.TileContext(nc) as tc:
        with tc.tile_pool(name="dram", bufs=1, space="DRAM") as dram:
            agin, agout = [], []
            for j in range(NCH):
                tlo, thi = _chunk(j)
                w = (thi - tlo) * BC
                agin.append(dram.tile([H, w], F16, name=f"agin{j}"))
                agout.append(dram.tile([NCORES, H, w], F16, name=f"agout{j}"))

            pwo_cm = tc.tile_pool(name="pwo", bufs=1)
            pwo = pwo_cm.__enter__()
            wo_all = pwo.tile([P, KH, VC], F16)
            hgat16 = pwo.tile([P, KH, NCH, P], F16)
            boutrep16 = pwo.tile([P, VC], F16)
            sums = pwo.tile([P, NCH, NV], F32)
            # only the last chunk's 96:128 pad rows are ever read unwritten
            nc.vector.memset(hgat16[:, :, NCH - 1, 96:P], 0.0)

            pfl_cm = tc.tile_pool(name="pfl", bufs=1)
            pfl = pfl_cm.__enter__()
            psfl_holder = {}
            ones16 = pfl.tile([1, P], F16)
            bout16 = pfl.tile([1, VC], F16)

            # ---- filler task machinery (vocab matmuls) ----
            tasks = [(j, n) for j in range(NCH) for n in range(NV)]
            ti = [0]
            pending = []

            def task_gate(j):
                if j < 6:
                    return TCH * j + 7
                if j == 6:
                    return 30
                return T + 1       # chunk 7 runs in the tail only

            def emit_filler(t):
                if ti[0] >= len(tasks):
                    return
                j, n = tasks[ti[0]]
                if t < task_gate(j):
                    return
                ti[0] += 1
                nsl = slice(n * NVS, (n + 1) * NVS)
                ps_o = psfl_holder["pool"].tile([P, NVS], F32, name="ps_o",
                                                tag="ps_o")
                for k in range(KH):
                    nc.tensor.matmul(ps_o[:], hgat16[:, k, j, :],
                                     wo_all[:, k, nsl],
                                     start=(k == 0), stop=(k == KH - 1))
                pending.append((j, n, ps_o))

            def flush_pending():
                for j, n, ps_o in pending:
                    nsl = slice(n * NVS, (n + 1) * NVS)
                    lgs = pfl.tile([P, NVS], F16, name="lgs", tag="lgs",
                                   bufs=4)
                    nc.vector.tensor_add(lgs[:], ps_o[:],
                                         boutrep16[:, nsl])
                    nc.sync.dma_start(out=out_d[j, :, nsl], in_=lgs[:])
                    etr = pfl.tile([P, NVS], F16, name="etr", tag="etr",
                                   bufs=2)
                    nc.scalar.activation(etr[:], lgs[:], AF.Exp,
                                         accum_out=sums[:, j, n:n + 1])
                pending.clear()

            with tc.tile_pool(name="pw", bufs=1) as pw:
                # ---- tiles that live through phases 0+1 ----
                whht = pw.tile([P, KH, 3 * H], F8)
                w1ht = pw.tile([P, KH, H], F8)
                ecT16 = pw.tile([P, KH, NBS], F16)
                encwc16 = pw.tile([P, 2, 3 * H], F8)
                gixt = pw.tile([P, KG, TS, BC], F16)
                hallT = pw.tile([P, KH, T, BC], F32)
                # two separate aw tiles: a single tile would create a
                # tile-granular WAR between half-1's stt write and the
                # half-0 e-matmul reads, serializing the attention pipeline
                awA = pw.tile([P, KH // 2, NBS], F16)
                awB = pw.tile([P, KH // 2, NBS], F16)
                w2t16 = pw.tile([P, KH], F16)
                b1t = pw.tile([P, KH], F32)
                bhnrep = pw.tile([P, KH, BC], F32)
                ones1 = pw.tile([1, 1], F16)
                bd1 = pw.tile([P, BC], F16)
                bd2 = pw.tile([P, BC], F16)

                nc.sync.dma_start(out=w2t16[:], in_=w2t16_d[:])
                nc.sync.dma_start(out=bout16[:], in_=bout16_d[:])
                nc.sync.dma_start(out=b1t[:], in_=b1t_d[:])
                nc.sync.dma_start(
                    out=bhnrep[:],
                    in_=bhnrep_d[:].rearrange("p (k b) -> p k b", b=BC))
                nc.sync.dma_start(
                    out=hallT[:, :, 0, :],
                    in_=h0t_d[:].rearrange("p (k b) -> p k b", b=BC))
                nc.vector.memset(ones1[:], 1.0)
                nc.vector.memset(bd1[:], 0.0)
                nc.vector.memset(bd2[:], 0.0)
                nc.vector.memset(awA[:], 0.0)
                nc.vector.memset(awB[:], 0.0)

                # ---------------- phase 0 ----------------
                with (
                    tc.tile_pool(name="p0b", bufs=1) as p0b,
                    tc.tile_pool(name="p0bs", bufs=2) as p0bs,
                ):
                    enct16 = p0b.tile([P, KH, NBS], F16)
                    nc.sync.dma_start(out=enct16[:], in_=enct16_d[:])

                    # EcT (k-outer, stream W1e per k; 8 live psum banks)
                    with tc.tile_pool(name="ps_ec_pool", bufs=1,
                                      space="PSUM") as psec:
                        ps_ec = [psec.tile([P, NBS], F32, name=f"ps_ec{mo}")
                                 for mo in range(KH)]
                        for k in range(KH):
                            w1ek = p0bs.tile([P, H], F8, name="w1ek", tag="w1ek")
                            nc.sync.dma_start(out=w1ek[:], in_=w1et16_d[:, k, :])
                            for mo in range(KH):
                                nc.tensor.matmul(
                                    ps_ec[mo][:], w1ek[:, mo * P:(mo + 1) * P],
                                    enct16[:, k, :],
                                    start=(k == 0), stop=(k == KH - 1))
                        for mo in range(KH):
                            nc.vector.scalar_tensor_tensor(
                                ecT16[:, mo, :], ps_ec[mo][:], WSI,
                                b1t[:, mo:mo + 1].broadcast_to([P, NBS]),
                                op0=ALU.mult, op1=ALU.add)

                    # step-1-critical recurrence weights ride the sync ring
                    # behind the (small) w1ek stream -- they land ~40us in,
                    # right when step 1 needs them
                    nc.sync.dma_start(out=w1ht[:], in_=w1ht_d[:])
                    nc.sync.dma_start(out=whht[:], in_=whht_d[:])

                    # EncWc (n-chunked 256 wide, stream WcT on scalar ring
                    # -- HWDGE, much lower per-transfer latency than SWDGE)
                    with tc.tile_pool(name="ps_ew_pool", bufs=2,
                                      space="PSUM") as psew:
                        for n in range(NWC):
                            wcs = p0bs.tile([P, KH, 256], F8, name="wcs",
                                            tag="wcs", bufs=4)
                            nc.scalar.dma_start(
                                out=wcs[:],
                                in_=wct16_d[:, :, n * 256:(n + 1) * 256])
                            for mt in range(2):
                                ps_ew = psew.tile([P, 256], F32, name="ps_ew",
                                                  tag="ps_ew")
                                for k in range(KH):
                                    nc.tensor.matmul(
                                        ps_ew[:],
                                        enct16[:, k, mt * P:(mt + 1) * P],
                                        wcs[:, k, :],
                                        start=(k == 0), stop=(k == KH - 1))
                                nc.vector.tensor_scalar(
                                    encwc16[:, mt, n * 256:(n + 1) * 256],
                                    ps_ew[:], WSI, None, op0=ALU.mult)

                # GIX (input projection for all steps; needed at t=1 gates)
                with (
                    tc.tile_pool(name="p0a", bufs=1) as p0a,
                    tc.tile_pool(name="p0as", bufs=2) as p0as,
                    tc.tile_pool(name="ps_gx_pool", bufs=1, space="PSUM") as psgx,
                ):
                    xat16 = p0a.tile([P, 5, P], F16)
                    nc.scalar.dma_start(out=xat16[:], in_=xat16_d[:])
                    wxb16 = p0a.tile([P, 3 * H], F16)
                    nc.scalar.dma_start(out=wxb16[:], in_=wxb16_d[:])
                    ps_gx = [psgx.tile([P, 4, P], F32, name=f"ps_gx{g}")
                             for g in range(6)]
                    for k in range(4):
                        # bufs=4: all four chunk DMAs issue up-front so the
                        # scalar ring never waits on GIX matmul pool slots
                        wxk = p0as.tile([P, 3 * H], F8, name="wxk", tag="wxk",
                                        bufs=4)
                        nc.scalar.dma_start(out=wxk[:], in_=wxat8_d[:, k, :])
                        for mo in range(KG):
                            nc.tensor.matmul(
                                ps_gx[mo // 4][:, mo % 4, :],
                                wxk[:, mo * P:(mo + 1) * P],
                                xat16[:, k, :], start=(k == 0), stop=False)
                    for mo in range(KG):
                        nc.tensor.matmul(
                            ps_gx[mo // 4][:, mo % 4, :],
                            wxb16[:, mo * P:(mo + 1) * P],
                            xat16[:, 4, :], start=False, stop=True)
                    for mo in range(KG):
                        nc.scalar.activation(
                            gixt[:, mo, :, :],
                            ps_gx[mo // 4][:, mo % 4, 0:TS * BC].rearrange(
                                "p (t b) -> p t b", b=BC),
                            AF.Copy, scale=WSI)

                # W_out is 8MB and only needed from t>=7; a real WAW dep on
                # a 1-element pre-write (which reads gixt) keeps the
                # scheduler from hoisting it ahead of the phase-0 streams
                woscr = pw.tile([1, 1], F16)
                nc.vector.tensor_copy(woscr[:], gixt[0:1, 0, 0, 0:1])
                nc.vector.tensor_copy(wo_all[0:1, 0, 0:1], woscr[:])
                nc.gpsimd.dma_start(out=wo_all[:], in_=woutt16_d[:])

                # filler psum pool opens once phase-0's 8-bank pools closed
                psfl_cm = tc.tile_pool(name="psfl", bufs=2, space="PSUM")
                psfl_holder["pool"] = psfl_cm.__enter__()

                nc.vector.memset(ones16[:], 1.0)

                # ---------------- phase 1: 31 steps ----------------
                with (
                    tc.tile_pool(name="p1", bufs=2) as p1,
                    tc.tile_pool(name="ps_hp_pool", bufs=1, space="PSUM") as pshp,
                    tc.tile_pool(name="ps_gh_pool", bufs=1, space="PSUM") as psgh,
                    tc.tile_pool(name="ps_gic_pool", bufs=1, space="PSUM") as psgic,
                    tc.tile_pool(name="ps_e_pool", bufs=1, space="PSUM") as pse,
                    tc.tile_pool(name="ps_a_pool", bufs=1, space="PSUM") as psa,
                ):
                    h16 = p1.tile([P, KH, BC], F16, name="h16", tag="h16")
                    nc.vector.tensor_copy(h16[:], hallT[:, :, 0, :])

                    for t in range(1, T):
                        hprev = hallT[:, :, t - 1, :]

                        if t == 3:
                            # b_out broadcast to all partitions via K=1 ones
                            # matmuls; placed here so it cannot delay the
                            # step-1-critical PE queue head
                            for n in range(NV):
                                ps_b = psfl_holder["pool"].tile(
                                    [P, NVS], F32, name="ps_b", tag="ps_o")
                                nc.tensor.matmul(
                                    ps_b[:], ones16[:],
                                    bout16[:, n * NVS:(n + 1) * NVS],
                                    start=True, stop=True)
                                nc.scalar.copy(
                                    boutrep16[:, n * NVS:(n + 1) * NVS],
                                    ps_b[:])

                        # Hproj (fp8 stationary, fp16 moving)
                        ps_hp = pshp.tile([P, KH, BC], F32, name="ps_hp",
                                          tag="hp")
                        for mo in range(KH):
                            for k in range(KH):
                                nc.tensor.matmul(
                                    ps_hp[:, mo, :],
                                    w1ht[:, k, mo * P:(mo + 1) * P],
                                    h16[:, k, :],
                                    start=(k == 0), stop=(k == KH - 1))

                        # attention: aw = tanh(EcT(+b1) + Hproj/WS), two
                        # independent tiles so the half-1 stt write cannot
                        # WAR-serialize against the half-0 e-matmul reads
                        KHH = KH // 2
                        stt_insts = []
                        for hh, aw in ((0, awA), (1, awB)):
                            ks = slice(hh * KHH, (hh + 1) * KHH)
                            stt_insts.append(nc.vector.scalar_tensor_tensor(
                                aw[:].rearrange(
                                    "p k (b s) -> p k b s", s=SP),
                                ps_hp[:, ks, :].broadcast_to(
                                    [P, KHH, BC, SP]),
                                WSI,
                                ecT16[:, ks, :].rearrange(
                                    "p k (b s) -> p k b s", s=SP),
                                op0=ALU.mult, op1=ALU.add))
                            nc.scalar.activation(aw[:], aw[:], AF.Tanh)

                        # gh (fp8 stationary, fp16 moving) -- the big one.
                        # Emitted in two halves with the e-score matmuls in
                        # between, so the scheduler slots e right after the
                        # aw tanh halves land and the softmax chain overlaps
                        # the gh tail.
                        ps_e = pse.tile([1, NBS], F32, name="ps_e", tag="e")
                        ps_gh = psgh.tile([P, KG, BC], F32, name="ps_gh",
                                          tag="gh")

                        def gh_block(mlo, mhi):
                            first = None
                            for mo in range(mlo, mhi):
                                for k in range(KH):
                                    mi = nc.tensor.matmul(
                                        ps_gh[:, mo, :],
                                        whht[:, k, mo * P:(mo + 1) * P],
                                        h16[:, k, :],
                                        start=(k == 0), stop=(k == KH - 1))
                                    if first is None:
                                        first = mi
                            return first

                        def e_block(klo, khi):
                            # ps_e accumulation group is interleaved with gh
                            # matmuls (different PSUM bank -- fine on HW)
                            last = None
                            for k in range(klo, khi):
                                aw = awA if k < KHH else awB
                                last = nc.tensor.matmul(
                                    ps_e[:], w2t16[:, k:k + 1],
                                    aw[:, k % KHH, :],
                                    start=(k == 0), stop=(k == KH - 1),
                                    skip_group_check=True)
                            return last

                        gh_block(0, 12)
                        ea = e_block(0, KHH)      # ready once tanh-A lands
                        g2 = gh_block(12, 18)
                        # flush here: prev step's filler epilogue lands in
                        # the gh window (ACT/vector/DMA idle)
                        flush_pending()
                        eb = e_block(KHH, KH)     # ready once tanh-B lands
                        g3 = gh_block(18, KG)

                        # hn only needs gh -- runs during the softmax window
                        hn = p1.tile([P, KH, BC], F32, name="hn", tag="hn")
                        nc.vector.scalar_tensor_tensor(
                            hn[:], ps_gh[:, 2 * KH:KG, :], WSI, bhnrep[:],
                            op0=ALU.mult, op1=ALU.add)

                        # softmax: direct Exp (same ACT table set as Tanh);
                        # no max-shift (|e| small).  1/sum is folded into
                        # the K=1 transpose matmuls via their rhs scalar.
                        expe = p1.tile([1, NBS], F16, name="expe", tag="expe")
                        nc.scalar.activation(expe[:], ps_e[:], AF.Exp)
                        s4 = p1.tile([1, BC], F32, name="s4", tag="s4")
                        nc.vector.reduce_sum(
                            s4[:], expe[:].rearrange("a (b s) -> a b s", s=SP)
                            [:, :, 0:S],
                            axis=mybir.AxisListType.X)
                        r4 = p1.tile([1, BC], F16, name="r4", tag="r4")
                        with nc.allow_low_precision(
                                reason="alpha normalization was fp16 anyway"):
                            nc.vector.reciprocal(r4[:], s4[:])

                        # transpose exp(e) to partitions, normalized on the
                        # fly: ps_a[64b:64b+64, c] = expe[b-block] * r4[b]
                        ps_a = psa.tile([P, 2], F32, name="ps_a", tag="a")
                        for bb in range(BC):
                            nc.tensor.matmul(
                                ps_a[(bb % 2) * 64:(bb % 2) * 64 + 64,
                                     bb // 2:bb // 2 + 1],
                                expe[:, bb * SP:(bb + 1) * SP],
                                r4[:, bb:bb + 1], start=True, stop=True)
                        # bd1 on vector, bd2 on scalar: the two pairs of
                        # block-diag copies run in parallel
                        nc.vector.tensor_copy(bd1[0:64, 0:1], ps_a[0:64, 0:1])
                        nc.vector.tensor_copy(bd1[64:128, 1:2],
                                              ps_a[64:128, 0:1])
                        nc.scalar.copy(bd2[0:64, 2:3], ps_a[0:64, 1:2])
                        nc.scalar.copy(bd2[64:128, 3:4],
                                       ps_a[64:128, 1:2])

                        # gi_c = blockdiag(alpha) applied to EncWc (fp16)
                        ps_gic = psgic.tile([P, KG, BC], F32, name="ps_gic",
                                            tag="gic")
                        for mo in range(KG):
                            nc.tensor.matmul(
                                ps_gic[:, mo, :],
                                encwc16[:, 0, mo * P:(mo + 1) * P],
                                bd1[:], start=True, stop=False)
                            nc.tensor.matmul(
                                ps_gic[:, mo, :],
                                encwc16[:, 1, mo * P:(mo + 1) * P],
                                bd2[:], start=False, stop=True)

                        emit_filler(t)   # gate-chain window fillers
                        emit_filler(t)

                        # gates, all-tanh: sig(x) = (1+tanh(x/2))/2
                        s1 = p1.tile([P, KG, BC], F32, name="s1", tag="s1")
                        nc.vector.tensor_add(s1[:], ps_gic[:],
                                             gixt[:, :, t - 1, :])
                        s2 = p1.tile([P, 2 * KH, BC], F32, name="s2", tag="s2")
                        nc.vector.scalar_tensor_tensor(
                            s2[:], ps_gh[:, 0:2 * KH, :], WSI,
                            s1[:, 0:2 * KH, :], op0=ALU.mult, op1=ALU.add)
                        trz = p1.tile([P, 2 * KH, BC], F32, name="trz",
                                      tag="trz")
                        nc.scalar.activation(trz[:], s2[:], AF.Tanh, scale=0.5)
                        # m1 = (trz_r + 1) * hn ; s3 = s1_n + 0.5*m1
                        m1 = p1.tile([P, KH, BC], F32, name="m1", tag="m1")
                        nc.vector.scalar_tensor_tensor(
                            m1[:], trz[:, 0:KH, :], 1.0, hn[:],
                            op0=ALU.add, op1=ALU.mult)
                        s3 = p1.tile([P, KH, BC], F32, name="s3", tag="s3")
                        nc.vector.scalar_tensor_tensor(
                            s3[:], m1[:], 0.5, s1[:, 2 * KH:KG, :],
                            op0=ALU.mult, op1=ALU.add)
                        nn_t = p1.tile([P, KH, BC], F32, name="nn_t", tag="nn")
                        nc.scalar.activation(nn_t[:], s3[:], AF.Tanh)
                        # h = nn + (trz_z + 1)/2 * (hprev - nn)
                        dd = p1.tile([P, KH, BC], F32, name="dd", tag="dd")
                        nc.vector.tensor_sub(dd[:], hprev, nn_t[:])
                        m2 = p1.tile([P, KH, BC], F32, name="m2", tag="m2")
                        nc.vector.scalar_tensor_tensor(
                            m2[:], trz[:, KH:2 * KH, :], 1.0, dd[:],
                            op0=ALU.add, op1=ALU.mult)
                        # fp16 h first (unblocks next-step matmuls), f32 second
                        h16 = p1.tile([P, KH, BC], F16, name="h16", tag="h16")
                        nc.vector.scalar_tensor_tensor(
                            h16[:], m2[:], 0.5, nn_t[:],
                            op0=ALU.mult, op1=ALU.add)
                        nc.vector.scalar_tensor_tensor(
                            hallT[:, :, t, :], m2[:], 0.5, nn_t[:],
                            op0=ALU.mult, op1=ALU.add)

                        emit_filler(t)   # catch-up slot

                        # stage this step's h into the AllGather input
                        jt = (t - 1) // TCH
                        tlo, thi = _chunk(jt)
                        nc.sync.dma_start(
                            out=agin[jt][:, :].rearrange(
                                "(k p) (tr b) -> p k tr b", p=P, b=BC)
                            [:, :, t - tlo, :],
                            in_=h16[:])
                        if t == thi - 1:
                            nc.gpsimd.collective_compute(
                                "AllGather", ALU.bypass,
                                replica_groups=rg,
                                ins=[agin[jt].opt()],
                                outs=[agout[jt].opt()])
                            if t < T - 1:
                                _gather_chunk(nc, agout, hgat16, jt)

                    # chunk NCH-1 lands after the loop
                    _gather_chunk(nc, agout, hgat16, NCH - 1)

                    # leftover vocab matmuls + exp accumulation
                    while ti[0] < len(tasks):
                        emit_filler(10 ** 9)
                        flush_pending()

                    # per-core expsums out (host adds across cores + log)
                    ssum = pfl.tile([P, NCH], F32, name="ssum")
                    nc.vector.reduce_sum(ssum[:], sums[:, :, :],
                                         axis=mybir.AxisListType.X)
                    nc.sync.dma_start(out=sums_d[:], in_=ssum[:])

            psfl_cm.__exit__(None, None, None)
            pfl_cm.__exit__(None, None, None)
            pwo_cm.__exit__(None, None, None)

    nc.compile()
    return nc


def _gather_chunk(nc, agout, hgat16, j):
    """DMA the gathered fp16 h slots of chunk j straight into
    hgat16[:, :, j, :].  Row order within the chunk is (rank, t, b)."""
    tlo, thi = _chunk(j)
    w = (thi - tlo) * BC
    for k in range(KH):
        nc.gpsimd.dma_start(
            out=hgat16[:, k, j, 0:NCORES * w].rearrange(
                "p (r w) -> p r w", r=NCORES),
            in_=agout[j][:, k * P:(k + 1) * P, :].rearrange("r p w -> p r w"))


def _t8(w, nk=8):
    # [nk*128, M] -> [128, nk, M]
    m = w.shape[1]
    return np.ascontiguousarray(w.reshape(nk, P, m).transpose(1, 0, 2))


def _prep_inputs(inputs):
    enc = np.asarray(inputs["encoder_outputs"], np.float32)
    ehid = np.asarray(inputs["encoder_hidden"], np.float32)
    targets = np.asarray(inputs["targets"])
    emb = np.asarray(inputs["emb"], np.float32)
    W1 = np.asarray(inputs["attn_W1"], np.float32)
    b1 = np.asarray(inputs["attn_b1"], np.float32)
    W2 = np.asarray(inputs["attn_W2"], np.float32)
    W_ih = np.asarray(inputs["W_ih"], np.float32)
    b_ih = np.asarray(inputs["b_ih"], np.float32)
    W_hh = np.asarray(inputs["W_hh"], np.float32)
    b_hh = np.asarray(inputs["b_hh"], np.float32)
    W_out = np.asarray(inputs["W_out"], np.float32)
    b_out = np.asarray(inputs["b_out"], np.float32)

    # shared (replicated across cores); big weights in fp8 (x64 scale)
    import ml_dtypes
    f8 = ml_dtypes.float8_e4m3fn
    w1et16 = (_t8(W1[:, :H].T) * 64).astype(f8)
    w1ht = (_t8(np.ascontiguousarray(W1[:, H:]).T) * 64).astype(f8)
    wct16 = (_t8(np.ascontiguousarray(W_ih[:, Dw:]).T) * 64).astype(f8)
    whht = (_t8(W_hh.T) * 64).astype(f8)
    wxa = np.zeros((640, 3 * H), np.float32)
    wxa[:Dw] = W_ih[:, :Dw].T
    wxa[Dw] = b_ih + np.concatenate([b_hh[:2 * H], np.zeros(H, np.float32)])
    wxat8 = (_t8(wxa[:512] * 64, nk=4)).astype(f8)
    wxb16 = (wxa[512:640] * 64).astype(np.float16)
    w2t16 = np.ascontiguousarray(W2[0].reshape(KH, P).T).astype(np.float16)
    b1t = np.ascontiguousarray(b1.reshape(KH, P).T)
    bhnrep = np.ascontiguousarray(
        np.repeat(b_hh[2 * H:].reshape(KH, P).T[:, :, None], BC, axis=2)
        .reshape(P, KH * BC))

    x_all = emb[targets[:, :TS]]  # [B, TS, Dw]

    in_maps = []
    for c in range(NCORES):
        bsl = slice(c * BC, (c + 1) * BC)
        vsl = slice(c * VC, (c + 1) * VC)
        encT = np.zeros((H, BC, SP), np.float32)
        encT[:, :, :S] = enc[bsl].transpose(2, 0, 1)
        enct16 = _t8(encT.reshape(H, NBS)).astype(np.float16)
        xat = np.zeros((640, P), np.float32)
        xat[:Dw, :TS * BC] = x_all[bsl].transpose(2, 1, 0).reshape(Dw, TS * BC)
        xat[Dw, :TS * BC] = 1.0
        xat16 = _t8(xat, nk=5).astype(np.float16)
        h0t = np.ascontiguousarray(
            ehid[0, bsl].T.reshape(KH, P, BC).transpose(1, 0, 2)
            .reshape(P, KH * BC))
        woutt16 = _t8(np.ascontiguousarray(W_out[vsl]).T).astype(np.float16)
        bout16 = np.ascontiguousarray(b_out[vsl][None, :]).astype(np.float16)
        in_maps.append({
            "enct16": enct16, "w1et16": w1et16, "wct16": wct16,
            "wxat8": wxat8, "wxb16": wxb16, "xat16": xat16, "whht": whht,
            "w1ht": w1ht, "w2t16": w2t16, "b1t": b1t, "bhnrep": bhnrep,
            "h0t": h0t, "woutt16": woutt16, "bout16": bout16,
        })
    return in_maps


def kernel(**inputs):
    if "nc" not in _CACHE:
        _CACHE["nc"] = _build()
    nc = _CACHE["nc"]
    in_maps = _prep_inputs(inputs)
    res = run_bass_kernel_spmd(nc, in_maps, core_ids=list(range(NCORES)))
    # out rows per chunk j are (rank, t, b); vocab sharded on cores.
    # Final log-softmax shift happens here: logZ = log(sum_cores expsum).
    L = np.stack([res.results[c]["out"] for c in range(NCORES)])
    Ssum = np.zeros((P, NCH), np.float64)
    for c in range(NCORES):
        Ssum += res.results[c]["sums"].astype(np.float64)
    logZ = np.log(Ssum).astype(np.float32).T        # [NCH, P]
    L = L.astype(np.float32)
    out = np.empty((B, TS, V), np.float32)
    for j in range(NCH):
        tlo, thi = _chunk(j)
        nt = thi - tlo
        rows = nt * B
        seg = L[:, j, :rows, :].reshape(NCORES, NCORES, nt, BC, VC)
        seg = seg - logZ[j, :rows].reshape(1, NCORES, nt, BC, 1)
        # [vcore, rank, t, b, vc] -> [rank*BC+b, t, vcore*VC+vc]
        out[:, tlo - 1:thi - 1, :] = (
            seg.transpose(1, 3, 2, 0, 4).reshape(B, nt, V))
    return out


# revision 35
# speedup vs baseline: 1.0175x; 1.0175x over previous
"""Attention-GRU decoder (teacher forcing) on 8 TRN2 NeuronCores.

Strategy (v6):
  Phase 0 (per core, batch sharded 4 seqs/core): precompute
     EcT    = (enc @ W1_enc.T).T + b1   -- attention enc projection
     EncWc  = enc @ Wc.T (fp8)          -- context->GRU-input projection
     GIX    = x_aug @ Wx_aug            -- all-steps input projection + biases
   Inputs stream on the three DMA-capable rings (sync/scalar/gpsimd);
   the 8MB W_out DMA carries a real WAW dep on a 1-element pre-write
   that reads gixt, so the scheduler cannot hoist it ahead of the
   phase-0-critical streams.
  Phase 1: 31 sequential steps, data-parallel over batch (BC=4/core).
     h @ W1h.T and h @ W_hh.T are fp8 stationary matmuls (LDW-bound,
     N=4 moving).  Attention softmax uses direct Exp (exp+tanh share the
     exp_and_others ACT table set, so no table reloads); gate sigmoids
     stay in tanh form.  e-score matmuls are emitted before the gh block
     so the scheduler runs them as soon as the aw tanh halves land,
     hiding the softmax chain under the gh matmul stream.  h is written
     fp16-first so next-step matmuls start immediately.  Partial fp16
     AllGathers of h every 4 steps (staged per-step) gather straight
     into the (t, rank, b) hgat buffer via strided DMA; vocab-projection
     matmuls fill the PE idle windows, their outputs (+b_out) are
     written directly to the output tensor and exp-summed on the fly.
  Tail: last AllGather + leftover vocab matmuls + exp accumulation,
     one [P, NCH] expsum output.  The final log-softmax shift
     (out -= log(sum_cores expsum)) is folded into the host-side
     unshard (all O(V) reduction work stays on-chip).

kernel(**inputs) takes full inputs, returns [B, T-1, V] float32.
"""
import numpy as np

import concourse.bacc as bacc
import concourse.bass as bass
import concourse.mybir as mybir
import concourse.tile as tile
from concourse.bass_utils import run_bass_kernel_spmd

F32 = mybir.dt.float32
F16 = mybir.dt.float16
F8 = mybir.dt.float8e4
AF = mybir.ActivationFunctionType
ALU = mybir.AluOpType
WS = 64.0            # fp8 weight scale
WSI = 1.0 / WS

B, S, H, V, Dw, T = 32, 50, 1024, 32000, 512, 32
NCORES = 8
P = 128
TS = T - 1            # 31 decode steps
BC = B // NCORES      # 4 sequences per core
VC = V // NCORES      # 4000 vocab rows per core
SP = 64               # padded s-block per sequence
NBS = BC * SP         # 256 padded (b,s) columns per core
KH = H // P           # 8 hidden chunks
KG = 3 * H // P       # 24 gate chunks
NV = 8                # vocab n-chunks per core
NVS = VC // NV        # 500
TCH = 4               # steps per AllGather chunk
NCH = 8               # number of chunks (last has 3 steps)
NWC = 12              # EncWc column chunks (256 wide)

_CACHE = {}


def _chunk(j):
    tlo = TCH * j + 1
    thi = min(tlo + TCH, T)
    return tlo, thi


def _build():
    nc = bacc.Bacc("TRN2", target_bir_lowering=False, debug=False,
                   num_devices=NCORES)

    def din(name, shape, dt):
        return nc.dram_tensor(name, shape, dt, kind="ExternalInput").ap()

    enct16_d = din("enct16", [P, KH, NBS], F16)
    w1et16_d = din("w1et16", [P, KH, H], F8)
    wct16_d = din("wct16", [P, KH, 3 * H], F8)
    wxat8_d = din("wxat8", [P, 4, 3 * H], F8)
    wxb16_d = din("wxb16", [P, 3 * H], F16)
    xat16_d = din("xat16", [P, 5, P], F16)
    whht_d = din("whht", [P, KH, 3 * H], F8)
    w1ht_d = din("w1ht", [P, KH, H], F8)
    w2t16_d = din("w2t16", [P, KH], F16)
    b1t_d = din("b1t", [P, KH], F32)
    bhnrep_d = din("bhnrep", [P, KH * BC], F32)
    h0t_d = din("h0t", [P, KH * BC], F32)
    woutt16_d = din("woutt16", [P, KH, VC], F16)
    bout16_d = din("bout16", [1, VC], F16)
    out_d = nc.dram_tensor("out", [NCH, P, VC], F16,
                           kind="ExternalOutput").ap()
    sums_d = nc.dram_tensor("sums", [P, NCH], F32,
                            kind="ExternalOutput").ap()

    rg = [list(range(NCORES))]

    with tile.TileContext(nc) as tc:
        with tc.tile_pool(name="dram", bufs=1, space="DRAM") as dram:
            agin, agout = [], []
            for j in range(NCH):
                tlo, thi = _chunk(j)
                w = (thi - tlo) * BC
                agin.append(dram.tile([H, w], F16, name=f"agin{j}"))
                agout.append(dram.tile([NCORES, H, w], F16, name=f"agout{j}"))

            pwo_cm = tc.tile_pool(name="pwo", bufs=1)
            pwo = pwo_cm.__enter__()
            wo_all = pwo.tile([P, KH, VC], F16)
            hgat16 = pwo.tile([P, KH, NCH, P], F16)
            boutrep16 = pwo.tile([P, VC], F16)
            sums = pwo.tile([P, NCH, NV], F32)
            # only the last chunk's 96:128 pad rows are ever read unwritten
            nc.vector.memset(hgat16[:, :, NCH - 1, 96:P], 0.0)

            pfl_cm = tc.tile_pool(name="pfl", bufs=1)
            pfl = pfl_cm.__enter__()
            psfl_holder = {}
            ones16 = pfl.tile([1, P], F16)
            bout16 = pfl.tile([1, VC], F16)

            # ---- filler task machinery (vocab matmuls) ----
            tasks = [(j, n) for j in range(NCH) for n in range(NV)]
            ti = [0]
            pending = []

            def task_gate(j):
                if j < 6:
                    return TCH * j + 7
                if j == 6:
                    return 30
                return T + 1       # chunk 7 runs in the tail only

            def emit_filler(t):
                if ti[0] >= len(tasks):
                    return
                j, n = tasks[ti[0]]
                if t < task_gate(j):
                    return
                ti[0] += 1
                nsl = slice(n * NVS, (n + 1) * NVS)
                ps_o = psfl_holder["pool"].tile([P, NVS], F32, name="ps_o",
                                                tag="ps_o")
                for k in range(KH):
                    nc.tensor.matmul(ps_o[:], hgat16[:, k, j, :],
                                     wo_all[:, k, nsl],
                                     start=(k == 0), stop=(k == KH - 1))
                pending.append((j, n, ps_o))

            def flush_pending():
                for j, n, ps_o in pending:
                    nsl = slice(n * NVS, (n + 1) * NVS)
                    lgs = pfl.tile([P, NVS], F16, name="lgs", tag="lgs",
                                   bufs=4)
                    nc.vector.tensor_add(lgs[:], ps_o[:],
                                         boutrep16[:, nsl])
                    nc.sync.dma_start(out=out_d[j, :, nsl], in_=lgs[:])
                    etr = pfl.tile([P, NVS], F16, name="etr", tag="etr",
                                   bufs=2)
                    nc.scalar.activation(etr[:], lgs[:], AF.Exp,
                                         accum_out=sums[:, j, n:n + 1])
                pending.clear()

            with tc.tile_pool(name="pw", bufs=1) as pw:
                # ---- tiles that live through phases 0+1 ----
                whht = pw.tile([P, KH, 3 * H], F8)
                w1ht = pw.tile([P, KH, H], F8)
                ecT16 = pw.tile([P, KH, NBS], F16)
                encwc16 = pw.tile([P, 2, 3 * H], F8)
                gixt = pw.tile([P, KG, TS, BC], F16)
                hallT = pw.tile([P, KH, T, BC], F32)
                # two separate aw tiles: a single tile would create a
                # tile-granular WAR between half-1's stt write and the
                # half-0 e-matmul reads, serializing the attention pipeline
                awA = pw.tile([P, KH // 2, NBS], F16)
                awB = pw.tile([P, KH // 2, NBS], F16)
                w2t16 = pw.tile([P, KH], F16)
                b1t = pw.tile([P, KH], F32)
                bhnrep = pw.tile([P, KH, BC], F32)
                ones1 = pw.tile([1, 1], F16)
                bd1 = pw.tile([P, BC], F16)
                bd2 = pw.tile([P, BC], F16)

                nc.sync.dma_start(out=w2t16[:], in_=w2t16_d[:])
                nc.sync.dma_start(out=bout16[:], in_=bout16_d[:])
                nc.sync.dma_start(out=b1t[:], in_=b1t_d[:])
                nc.sync.dma_start(
                    out=bhnrep[:],
                    in_=bhnrep_d[:].rearrange("p (k b) -> p k b", b=BC))
                nc.sync.dma_start(
                    out=hallT[:, :, 0, :],
                    in_=h0t_d[:].rearrange("p (k b) -> p k b", b=BC))
                nc.vector.memset(ones1[:], 1.0)
                nc.vector.memset(bd1[:], 0.0)
                nc.vector.memset(bd2[:], 0.0)
                nc.vector.memset(awA[:], 0.0)
                nc.vector.memset(awB[:], 0.0)

                # ---------------- phase 0 ----------------
                with (
                    tc.tile_pool(name="p0b", bufs=1) as p0b,
                    tc.tile_pool(name="p0bs", bufs=2) as p0bs,
                ):
                    enct16 = p0b.tile([P, KH, NBS], F16)
                    nc.sync.dma_start(out=enct16[:], in_=enct16_d[:])

                    # EcT (k-outer, stream W1e per k; 8 live psum banks)
                    with tc.tile_pool(name="ps_ec_pool", bufs=1,
                                      space="PSUM") as psec:
                        ps_ec = [psec.tile([P, NBS], F32, name=f"ps_ec{mo}")
                                 for mo in range(KH)]
                        for k in range(KH):
                            w1ek = p0bs.tile([P, H], F8, name="w1ek", tag="w1ek")
                            nc.sync.dma_start(out=w1ek[:], in_=w1et16_d[:, k, :])
                            for mo in range(KH):
                                nc.tensor.matmul(
                                    ps_ec[mo][:], w1ek[:, mo * P:(mo + 1) * P],
                                    enct16[:, k, :],
                                    start=(k == 0), stop=(k == KH - 1))
                        for mo in range(KH):
                            nc.vector.scalar_tensor_tensor(
                                ecT16[:, mo, :], ps_ec[mo][:], WSI,
                                b1t[:, mo:mo + 1].broadcast_to([P, NBS]),
                                op0=ALU.mult, op1=ALU.add)

                    # EncWc (n-chunked 256 wide, stream WcT on gpsimd ring)
                    with tc.tile_pool(name="ps_ew_pool", bufs=2,
                                      space="PSUM") as psew:
                        for n in range(NWC):
                            wcs = p0bs.tile([P, KH, 256], F8, name="wcs",
                                            tag="wcs", bufs=4)
                            nc.gpsimd.dma_start(
                                out=wcs[:],
                                in_=wct16_d[:, :, n * 256:(n + 1) * 256])
                            for mt in range(2):
                                ps_ew = psew.tile([P, 256], F32, name="ps_ew",
                                                  tag="ps_ew")
                                for k in range(KH):
                                    nc.tensor.matmul(
                                        ps_ew[:],
                                        enct16[:, k, mt * P:(mt + 1) * P],
                                        wcs[:, k, :],
                                        start=(k == 0), stop=(k == KH - 1))
                                nc.vector.tensor_scalar(
                                    encwc16[:, mt, n * 256:(n + 1) * 256],
                                    ps_ew[:], WSI, None, op0=ALU.mult)

                # GIX (input projection for all steps; needed at t=1 gates)
                with (
                    tc.tile_pool(name="p0a", bufs=1) as p0a,
                    tc.tile_pool(name="p0as", bufs=2) as p0as,
                    tc.tile_pool(name="ps_gx_pool", bufs=1, space="PSUM") as psgx,
                ):
                    xat16 = p0a.tile([P, 5, P], F16)
                    nc.scalar.dma_start(out=xat16[:], in_=xat16_d[:])
                    wxb16 = p0a.tile([P, 3 * H], F16)
                    nc.scalar.dma_start(out=wxb16[:], in_=wxb16_d[:])
                    ps_gx = [psgx.tile([P, 4, P], F32, name=f"ps_gx{g}")
                             for g in range(6)]
                    for k in range(4):
                        wxk = p0as.tile([P, 3 * H], F8, name="wxk", tag="wxk")
                        nc.scalar.dma_start(out=wxk[:], in_=wxat8_d[:, k, :])
                        for mo in range(KG):
                            nc.tensor.matmul(
                                ps_gx[mo // 4][:, mo % 4, :],
                                wxk[:, mo * P:(mo + 1) * P],
                                xat16[:, k, :], start=(k == 0), stop=False)
                    # step-1-critical recurrence weights follow on the
                    # scalar ring (arrive ~when phase 1 starts)
                    nc.scalar.dma_start(out=w1ht[:], in_=w1ht_d[:])
                    nc.scalar.dma_start(out=whht[:], in_=whht_d[:])
                    for mo in range(KG):
                        nc.tensor.matmul(
                            ps_gx[mo // 4][:, mo % 4, :],
                            wxb16[:, mo * P:(mo + 1) * P],
                            xat16[:, 4, :], start=False, stop=True)
                    for mo in range(KG):
                        nc.scalar.activation(
                            gixt[:, mo, :, :],
                            ps_gx[mo // 4][:, mo % 4, 0:TS * BC].rearrange(
                                "p (t b) -> p t b", b=BC),
                            AF.Copy, scale=WSI)

                # W_out is 8MB and only needed from t>=7; a real WAW dep on
                # a 1-element pre-write (which reads gixt) keeps the
                # scheduler from hoisting it ahead of the phase-0 streams
                woscr = pw.tile([1, 1], F16)
                nc.vector.tensor_copy(woscr[:], gixt[0:1, 0, 0, 0:1])
                nc.vector.tensor_copy(wo_all[0:1, 0, 0:1], woscr[:])
                nc.gpsimd.dma_start(out=wo_all[:], in_=woutt16_d[:])

                # filler psum pool opens once phase-0's 8-bank pools closed
                psfl_cm = tc.tile_pool(name="psfl", bufs=2, space="PSUM")
                psfl_holder["pool"] = psfl_cm.__enter__()

                nc.vector.memset(ones16[:], 1.0)

                # ---------------- phase 1: 31 steps ----------------
                with (
                    tc.tile_pool(name="p1", bufs=2) as p1,
                    tc.tile_pool(name="ps_hp_pool", bufs=1, space="PSUM") as pshp,
                    tc.tile_pool(name="ps_gh_pool", bufs=1, space="PSUM") as psgh,
                    tc.tile_pool(name="ps_gic_pool", bufs=1, space="PSUM") as psgic,
                    tc.tile_pool(name="ps_e_pool", bufs=1, space="PSUM") as pse,
                    tc.tile_pool(name="ps_a_pool", bufs=1, space="PSUM") as psa,
                ):
                    h16 = p1.tile([P, KH, BC], F16, name="h16", tag="h16")
                    nc.vector.tensor_copy(h16[:], hallT[:, :, 0, :])

                    for t in range(1, T):
                        hprev = hallT[:, :, t - 1, :]

                        if t == 3:
                            # b_out broadcast to all partitions via K=1 ones
                            # matmuls; placed here so it cannot delay the
                            # step-1-critical PE queue head
                            for n in range(NV):
                                ps_b = psfl_holder["pool"].tile(
                                    [P, NVS], F32, name="ps_b", tag="ps_o")
                                nc.tensor.matmul(
                                    ps_b[:], ones16[:],
                                    bout16[:, n * NVS:(n + 1) * NVS],
                                    start=True, stop=True)
                                nc.scalar.copy(
                                    boutrep16[:, n * NVS:(n + 1) * NVS],
                                    ps_b[:])

                        # Hproj (fp8 stationary, fp16 moving)
                        ps_hp = pshp.tile([P, KH, BC], F32, name="ps_hp",
                                          tag="hp")
                        for mo in range(KH):
                            for k in range(KH):
                                nc.tensor.matmul(
                                    ps_hp[:, mo, :],
                                    w1ht[:, k, mo * P:(mo + 1) * P],
                                    h16[:, k, :],
                                    start=(k == 0), stop=(k == KH - 1))

                        # attention: aw = tanh(EcT(+b1) + Hproj/WS), two
                        # independent tiles so the half-1 stt write cannot
                        # WAR-serialize against the half-0 e-matmul reads
                        KHH = KH // 2
                        stt_insts = []
                        for hh, aw in ((0, awA), (1, awB)):
                            ks = slice(hh * KHH, (hh + 1) * KHH)
                            stt_insts.append(nc.vector.scalar_tensor_tensor(
                                aw[:].rearrange(
                                    "p k (b s) -> p k b s", s=SP),
                                ps_hp[:, ks, :].broadcast_to(
                                    [P, KHH, BC, SP]),
                                WSI,
                                ecT16[:, ks, :].rearrange(
                                    "p k (b s) -> p k b s", s=SP),
                                op0=ALU.mult, op1=ALU.add))
                            nc.scalar.activation(aw[:], aw[:], AF.Tanh)

                        # gh (fp8 stationary, fp16 moving) -- the big one.
                        # Emitted in two halves with the e-score matmuls in
                        # between, so the scheduler slots e right after the
                        # aw tanh halves land and the softmax chain overlaps
                        # the gh tail.
                        ps_e = pse.tile([1, NBS], F32, name="ps_e", tag="e")
                        ps_gh = psgh.tile([P, KG, BC], F32, name="ps_gh",
                                          tag="gh")

                        def gh_block(mlo, mhi):
                            first = None
                            for mo in range(mlo, mhi):
                                for k in range(KH):
                                    mi = nc.tensor.matmul(
                                        ps_gh[:, mo, :],
                                        whht[:, k, mo * P:(mo + 1) * P],
                                        h16[:, k, :],
                                        start=(k == 0), stop=(k == KH - 1))
                                    if first is None:
                                        first = mi
                            return first

                        def e_block(klo, khi):
                            # ps_e accumulation group is interleaved with gh
                            # matmuls (different PSUM bank -- fine on HW)
                            last = None
                            for k in range(klo, khi):
                                aw = awA if k < KHH else awB
                                last = nc.tensor.matmul(
                                    ps_e[:], w2t16[:, k:k + 1],
                                    aw[:, k % KHH, :],
                                    start=(k == 0), stop=(k == KH - 1),
                                    skip_group_check=True)
                            return last

                        gh_block(0, 12)
                        ea = e_block(0, KHH)      # ready once tanh-A lands
                        g2 = gh_block(12, 18)
                        # flush here: prev step's filler epilogue lands in
                        # the gh window (ACT/vector/DMA idle)
                        flush_pending()
                        eb = e_block(KHH, KH)     # ready once tanh-B lands
                        g3 = gh_block(18, KG)

                        # hn only needs gh -- runs during the softmax window
                        hn = p1.tile([P, KH, BC], F32, name="hn", tag="hn")
                        nc.vector.scalar_tensor_tensor(
                            hn[:], ps_gh[:, 2 * KH:KG, :], WSI, bhnrep[:],
                            op0=ALU.mult, op1=ALU.add)

                        # softmax: direct Exp (same ACT table set as Tanh);
                        # no max-shift (|e| small).  1/sum is folded into
                        # the K=1 transpose matmuls via their rhs scalar.
                        expe = p1.tile([1, NBS], F16, name="expe", tag="expe")
                        nc.scalar.activation(expe[:], ps_e[:], AF.Exp)
                        s4 = p1.tile([1, BC], F32, name="s4", tag="s4")
                        nc.vector.reduce_sum(
                            s4[:], expe[:].rearrange("a (b s) -> a b s", s=SP)
                            [:, :, 0:S],
                            axis=mybir.AxisListType.X)
                        r4 = p1.tile([1, BC], F16, name="r4", tag="r4")
                        with nc.allow_low_precision(
                                reason="alpha normalization was fp16 anyway"):
                            nc.vector.reciprocal(r4[:], s4[:])

                        # transpose exp(e) to partitions, normalized on the
                        # fly: ps_a[64b:64b+64, c] = expe[b-block] * r4[b]
                        ps_a = psa.tile([P, 2], F32, name="ps_a", tag="a")
                        for bb in range(BC):
                            nc.tensor.matmul(
                                ps_a[(bb % 2) * 64:(bb % 2) * 64 + 64,
                                     bb // 2:bb // 2 + 1],
                                expe[:, bb * SP:(bb + 1) * SP],
                                r4[:, bb:bb + 1], start=True, stop=True)
                        # bd1 on vector, bd2 on scalar: the two pairs of
                        # block-diag copies run in parallel
                        nc.vector.tensor_copy(bd1[0:64, 0:1], ps_a[0:64, 0:1])
                        nc.vector.tensor_copy(bd1[64:128, 1:2],
                                              ps_a[64:128, 0:1])
                        nc.scalar.copy(bd2[0:64, 2:3], ps_a[0:64, 1:2])
                        nc.scalar.copy(bd2[64:128, 3:4],
                                       ps_a[64:128, 1:2])

                        # gi_c = blockdiag(alpha) applied to EncWc (fp16)
                        ps_gic = psgic.tile([P, KG, BC], F32, name="ps_gic",
                                            tag="gic")
                        for mo in range(KG):
                            nc.tensor.matmul(
                                ps_gic[:, mo, :],
                                encwc16[:, 0, mo * P:(mo + 1) * P],
                                bd1[:], start=True, stop=False)
                            nc.tensor.matmul(
                                ps_gic[:, mo, :],
                                encwc16[:, 1, mo * P:(mo + 1) * P],
                                bd2[:], start=False, stop=True)

                        emit_filler(t)   # gate-chain window fillers
                        emit_filler(t)

                        # gates, all-tanh: sig(x) = (1+tanh(x/2))/2
                        s1 = p1.tile([P, KG, BC], F32, name="s1", tag="s1")
                        nc.vector.tensor_add(s1[:], ps_gic[:],
                                             gixt[:, :, t - 1, :])
                        s2 = p1.tile([P, 2 * KH, BC], F32, name="s2", tag="s2")
                        nc.vector.scalar_tensor_tensor(
                            s2[:], ps_gh[:, 0:2 * KH, :], WSI,
                            s1[:, 0:2 * KH, :], op0=ALU.mult, op1=ALU.add)
                        trz = p1.tile([P, 2 * KH, BC], F32, name="trz",
                                      tag="trz")
                        nc.scalar.activation(trz[:], s2[:], AF.Tanh, scale=0.5)
                        # m1 = (trz_r + 1) * hn ; s3 = s1_n + 0.5*m1
                        m1 = p1.tile([P, KH, BC], F32, name="m1", tag="m1")
                        nc.vector.scalar_tensor_tensor(
                            m1[:], trz[:, 0:KH, :], 1.0, hn[:],
                            op0=ALU.add, op1=ALU.mult)
                        s3 = p1.tile([P, KH, BC], F32, name="s3", tag="s3")
                        nc.vector.scalar_tensor_tensor(
                            s3[:], m1[:], 0.5, s1[:, 2 * KH:KG, :],
                            op0=ALU.mult, op1=ALU.add)
                        nn_t = p1.tile([P, KH, BC], F32, name="nn_t", tag="nn")
                        nc.scalar.activation(nn_t[:], s3[:], AF.Tanh)
                        # h = nn + (trz_z + 1)/2 * (hprev - nn)
                        dd = p1.tile([P, KH, BC], F32, name="dd", tag="dd")
                        nc.vector.tensor_sub(dd[:], hprev, nn_t[:])
                        m2 = p1.tile([P, KH, BC], F32, name="m2", tag="m2")
                        nc.vector.scalar_tensor_tensor(
                            m2[:], trz[:, KH:2 * KH, :], 1.0, dd[:],
                            op0=ALU.add, op1=ALU.mult)
                        # fp16 h first (unblocks next-step matmuls), f32 second
                        h16 = p1.tile([P, KH, BC], F16, name="h16", tag="h16")
                        nc.vector.scalar_tensor_tensor(
                            h16[:], m2[:], 0.5, nn_t[:],
                            op0=ALU.mult, op1=ALU.add)
                        nc.vector.scalar_tensor_tensor(
                            hallT[:, :, t, :], m2[:], 0.5, nn_t[:],
                            op0=ALU.mult, op1=ALU.add)

                        emit_filler(t)   # catch-up slot

                        # stage this step's h into the AllGather input
                        jt = (t - 1) // TCH
                        tlo, thi = _chunk(jt)
                        nc.sync.dma_start(
                            out=agin[jt][:, :].rearrange(
                                "(k p) (tr b) -> p k tr b", p=P, b=BC)
                            [:, :, t - tlo, :],
                            in_=h16[:])
                        if t == thi - 1:
                            nc.gpsimd.collective_compute(
                                "AllGather", ALU.bypass,
                                replica_groups=rg,
                                ins=[agin[jt].opt()],
                                outs=[agout[jt].opt()])
                            if t < T - 1:
                                _gather_chunk(nc, agout, hgat16, jt)

                    # chunk NCH-1 lands after the loop
                    _gather_chunk(nc, agout, hgat16, NCH - 1)

                    # leftover vocab matmuls + exp accumulation
                    while ti[0] < len(tasks):
                        emit_filler(10 ** 9)
                        flush_pending()

                    # per-core expsums out (host adds across cores + log)
                    ssum = pfl.tile([P, NCH], F32, name="ssum")
                    nc.vector.reduce_sum(ssum[:], sums[:, :, :],
                                         axis=mybir.AxisListType.X)
                    nc.sync.dma_start(out=sums_d[:], in_=ssum[:])

            psfl_cm.__exit__(None, None, None)
            pfl_cm.__exit__(None, None, None)
            pwo_cm.__exit__(None, None, None)

    nc.compile()
    return nc


def _gather_chunk(nc, agout, hgat16, j):
    """DMA the gathered fp16 h slots of chunk j straight into
    hgat16[:, :, j, :].  Row order within the chunk is (rank, t, b)."""
    tlo, thi = _chunk(j)
    w = (thi - tlo) * BC
    for k in range(KH):
        nc.gpsimd.dma_start(
            out=hgat16[:, k, j, 0:NCORES * w].rearrange(
                "p (r w) -> p r w", r=NCORES),
            in_=agout[j][:, k * P:(k + 1) * P, :].rearrange("r p w -> p r w"))


def _t8(w, nk=8):
    # [nk*128, M] -> [128, nk, M]
    m = w.shape[1]
    return np.ascontiguousarray(w.reshape(nk, P, m).transpose(1, 0, 2))


def _prep_inputs(inputs):
    enc = np.asarray(inputs["encoder_outputs"], np.float32)
    ehid = np.asarray(inputs["encoder_hidden"], np.float32)
    targets = np.asarray(inputs["targets"])
    emb = np.asarray(inputs["emb"], np.float32)
    W1 = np.asarray(inputs["attn_W1"], np.float32)
    b1 = np.asarray(inputs["attn_b1"], np.float32)
    W2 = np.asarray(inputs["attn_W2"], np.float32)
    W_ih = np.asarray(inputs["W_ih"], np.float32)
    b_ih = np.asarray(inputs["b_ih"], np.float32)
    W_hh = np.asarray(inputs["W_hh"], np.float32)
    b_hh = np.asarray(inputs["b_hh"], np.float32)
    W_out = np.asarray(inputs["W_out"], np.float32)
    b_out = np.asarray(inputs["b_out"], np.float32)

    # shared (replicated across cores); big weights in fp8 (x64 scale)
    import ml_dtypes
    f8 = ml_dtypes.float8_e4m3fn
    w1et16 = (_t8(W1[:, :H].T) * 64).astype(f8)
    w1ht = (_t8(np.ascontiguousarray(W1[:, H:]).T) * 64).astype(f8)
    wct16 = (_t8(np.ascontiguousarray(W_ih[:, Dw:]).T) * 64).astype(f8)
    whht = (_t8(W_hh.T) * 64).astype(f8)
    wxa = np.zeros((640, 3 * H), np.float32)
    wxa[:Dw] = W_ih[:, :Dw].T
    wxa[Dw] = b_ih + np.concatenate([b_hh[:2 * H], np.zeros(H, np.float32)])
    wxat8 = (_t8(wxa[:512] * 64, nk=4)).astype(f8)
    wxb16 = (wxa[512:640] * 64).astype(np.float16)
    w2t16 = np.ascontiguousarray(W2[0].reshape(KH, P).T).astype(np.float16)
    b1t = np.ascontiguousarray(b1.reshape(KH, P).T)
    bhnrep = np.ascontiguousarray(
        np.repeat(b_hh[2 * H:].reshape(KH, P).T[:, :, None], BC, axis=2)
        .reshape(P, KH * BC))

    x_all = emb[targets[:, :TS]]  # [B, TS, Dw]

    in_maps = []
    for c in range(NCORES):
        bsl = slice(c * BC, (c + 1) * BC)
        vsl = slice(c * VC, (c + 1) * VC)
        encT = np.zeros((H, BC, SP), np.float32)
        encT[:, :, :S] = enc[bsl].transpose(2, 0, 1)
        enct16 = _t8(encT.reshape(H, NBS)).astype(np.float16)
        xat = np.zeros((640, P), np.float32)
        xat[:Dw, :TS * BC] = x_all[bsl].transpose(2, 1, 0).reshape(Dw, TS * BC)
        xat[Dw, :TS * BC] = 1.0
        xat16 = _t8(xat, nk=5).astype(np.float16)
        h0t = np.ascontiguousarray(
            ehid[0, bsl].T.reshape(KH, P, BC).transpose(1, 0, 2)
            .reshape(P, KH * BC))
        woutt16 = _t8(np.ascontiguousarray(W_out[vsl]).T).astype(np.float16)
        bout16 = np.ascontiguousarray(b_out[vsl][None, :]).astype(np.float16)
        in_maps.append({
            "enct16": enct16, "w1et16": w1et16, "wct16": wct16,
            "wxat8": wxat8, "wxb16": wxb16, "xat16": xat16, "whht": whht,
            "w1ht": w1ht, "w2t16": w2t16, "b1t": b1t, "bhnrep": bhnrep,
            "h0t": h0t, "woutt16": woutt16, "bout16": bout16,
        })
    return in_maps


def kernel(**inputs):
    if "nc" not in _CACHE:
        _CACHE["nc"] = _build()
    nc = _CACHE["nc"]
    in_maps = _prep_inputs(inputs)
    res = run_bass_kernel_spmd(nc, in_maps, core_ids=list(range(NCORES)))
    # out rows per chunk j are (rank, t, b); vocab sharded on cores.
    # Final log-softmax shift happens here: logZ = log(sum_cores expsum).
    L = np.stack([res.results[c]["out"] for c in range(NCORES)])
    Ssum = np.zeros((P, NCH), np.float64)
    for c in range(NCORES):
        Ssum += res.results[c]["sums"].astype(np.float64)
    logZ = np.log(Ssum).astype(np.float32).T        # [NCH, P]
    L = L.astype(np.float32)
    out = np.empty((B, TS, V), np.float32)
    for j in range(NCH):
        tlo, thi = _chunk(j)
        nt = thi - tlo
        rows = nt * B
        seg = L[:, j, :rows, :].reshape(NCORES, NCORES, nt, BC, VC)
        seg = seg - logZ[j, :rows].reshape(1, NCORES, nt, BC, 1)
        # [vcore, rank, t, b, vc] -> [rank*BC+b, t, vcore*VC+vc]
        out[:, tlo - 1:thi - 1, :] = (
            seg.transpose(1, 3, 2, 0, 4).reshape(B, nt, V))
    return out


# revision 38
# speedup vs baseline: 1.0890x; 1.0703x over previous
"""Attention-GRU decoder (teacher forcing) on 8 TRN2 NeuronCores.

Strategy (v6):
  Phase 0 (per core, batch sharded 4 seqs/core): precompute
     EcT    = (enc @ W1_enc.T).T + b1   -- attention enc projection
     EncWc  = enc @ Wc.T (fp8)          -- context->GRU-input projection
     GIX    = x_aug @ Wx_aug            -- all-steps input projection + biases
   Inputs stream on the three DMA-capable rings (sync/scalar/gpsimd);
   the 8MB W_out DMA carries a real WAW dep on a 1-element pre-write
   that reads gixt, so the scheduler cannot hoist it ahead of the
   phase-0-critical streams.
  Phase 1: 31 sequential steps, data-parallel over batch (BC=4/core).
     h @ W1h.T and h @ W_hh.T are fp8 stationary matmuls (LDW-bound,
     N=4 moving).  Attention softmax uses direct Exp (exp+tanh share the
     exp_and_others ACT table set, so no table reloads); gate sigmoids
     stay in tanh form.  e-score matmuls are emitted before the gh block
     so the scheduler runs them as soon as the aw tanh halves land,
     hiding the softmax chain under the gh matmul stream.  h is written
     fp16-first so next-step matmuls start immediately.  Partial fp16
     AllGathers of h every 4 steps (staged per-step) gather straight
     into the (t, rank, b) hgat buffer via strided DMA; vocab-projection
     matmuls fill the PE idle windows, their outputs (+b_out) are
     written directly to the output tensor and exp-summed on the fly.
  Tail: last AllGather + leftover vocab matmuls + exp accumulation,
     one [P, NCH] expsum output.  The final log-softmax shift
     (out -= log(sum_cores expsum)) is folded into the host-side
     unshard (all O(V) reduction work stays on-chip).

kernel(**inputs) takes full inputs, returns [B, T-1, V] float32.
"""
import numpy as np

import concourse.bacc as bacc
import concourse.bass as bass
import concourse.mybir as mybir
import concourse.tile as tile
from concourse.bass_utils import run_bass_kernel_spmd

F32 = mybir.dt.float32
F16 = mybir.dt.float16
F8 = mybir.dt.float8e4
AF = mybir.ActivationFunctionType
ALU = mybir.AluOpType
WS = 64.0            # fp8 weight scale
WSI = 1.0 / WS

B, S, H, V, Dw, T = 32, 50, 1024, 32000, 512, 32
NCORES = 8
P = 128
TS = T - 1            # 31 decode steps
BC = B // NCORES      # 4 sequences per core
VC = V // NCORES      # 4000 vocab rows per core
SP = 64               # padded s-block per sequence
NBS = BC * SP         # 256 padded (b,s) columns per core
KH = H // P           # 8 hidden chunks
KG = 3 * H // P       # 24 gate chunks
NV = 8                # vocab n-chunks per core
NVS = VC // NV        # 500
TCH = 4               # steps per AllGather chunk
NCH = 8               # number of chunks (last has 3 steps)
NWC = 12              # EncWc column chunks (256 wide)

_CACHE = {}


def _chunk(j):
    tlo = TCH * j + 1
    thi = min(tlo + TCH, T)
    return tlo, thi


def _build():
    nc = bacc.Bacc("TRN2", target_bir_lowering=False, debug=False,
                   num_devices=NCORES)

    def din(name, shape, dt):
        return nc.dram_tensor(name, shape, dt, kind="ExternalInput").ap()

    enct16_d = din("enct16", [P, KH, NBS], F16)
    w1et16_d = din("w1et16", [P, KH, H], F8)
    wct16_d = din("wct16", [P, KH, 3 * H], F8)
    wxat8_d = din("wxat8", [P, 4, 3 * H], F8)
    wxb16_d = din("wxb16", [P, 3 * H], F16)
    xat16_d = din("xat16", [P, 5, P], F16)
    whht_d = din("whht", [P, KH, 3 * H], F8)
    w1ht_d = din("w1ht", [P, KH, H], F8)
    w2t16_d = din("w2t16", [P, KH], F16)
    b1t_d = din("b1t", [P, KH], F32)
    bhnrep_d = din("bhnrep", [P, KH * BC], F32)
    h0t_d = din("h0t", [P, KH * BC], F32)
    woutt16_d = din("woutt16", [P, KH, VC], F16)
    bout16_d = din("bout16", [1, VC], F16)
    out_d = nc.dram_tensor("out", [NCH, P, VC], F16,
                           kind="ExternalOutput").ap()
    sums_d = nc.dram_tensor("sums", [P, NCH], F32,
                            kind="ExternalOutput").ap()

    rg = [list(range(NCORES))]

    with tile.TileContext(nc) as tc:
        with tc.tile_pool(name="dram", bufs=1, space="DRAM") as dram:
            agin, agout = [], []
            for j in range(NCH):
                tlo, thi = _chunk(j)
                w = (thi - tlo) * BC
                agin.append(dram.tile([H, w], F16, name=f"agin{j}"))
                agout.append(dram.tile([NCORES, H, w], F16, name=f"agout{j}"))

            pwo_cm = tc.tile_pool(name="pwo", bufs=1)
            pwo = pwo_cm.__enter__()
            wo_all = pwo.tile([P, KH, VC], F16)
            hgat16 = pwo.tile([P, KH, NCH, P], F16)
            boutrep16 = pwo.tile([P, VC], F16)
            sums = pwo.tile([P, NCH, NV], F32)
            # only the last chunk's 96:128 pad rows are ever read unwritten
            nc.vector.memset(hgat16[:, :, NCH - 1, 96:P], 0.0)

            pfl_cm = tc.tile_pool(name="pfl", bufs=1)
            pfl = pfl_cm.__enter__()
            psfl_holder = {}
            ones16 = pfl.tile([1, P], F16)
            bout16 = pfl.tile([1, VC], F16)

            # ---- filler task machinery (vocab matmuls) ----
            tasks = [(j, n) for j in range(NCH) for n in range(NV)]
            ti = [0]
            pending = []

            def task_gate(j):
                if j < 6:
                    return TCH * j + 7
                if j == 6:
                    return 30
                return T + 1       # chunk 7 runs in the tail only

            def emit_filler(t):
                if ti[0] >= len(tasks):
                    return
                j, n = tasks[ti[0]]
                if t < task_gate(j):
                    return
                ti[0] += 1
                nsl = slice(n * NVS, (n + 1) * NVS)
                ps_o = psfl_holder["pool"].tile([P, NVS], F32, name="ps_o",
                                                tag="ps_o")
                for k in range(KH):
                    nc.tensor.matmul(ps_o[:], hgat16[:, k, j, :],
                                     wo_all[:, k, nsl],
                                     start=(k == 0), stop=(k == KH - 1))
                pending.append((j, n, ps_o))

            def flush_pending():
                for j, n, ps_o in pending:
                    nsl = slice(n * NVS, (n + 1) * NVS)
                    lgs = pfl.tile([P, NVS], F16, name="lgs", tag="lgs",
                                   bufs=4)
                    nc.vector.tensor_add(lgs[:], ps_o[:],
                                         boutrep16[:, nsl])
                    nc.sync.dma_start(out=out_d[j, :, nsl], in_=lgs[:])
                    etr = pfl.tile([P, NVS], F16, name="etr", tag="etr",
                                   bufs=2)
                    nc.scalar.activation(etr[:], lgs[:], AF.Exp,
                                         accum_out=sums[:, j, n:n + 1])
                pending.clear()

            with tc.tile_pool(name="pw", bufs=1) as pw:
                # ---- tiles that live through phases 0+1 ----
                whht = pw.tile([P, KH, 3 * H], F8)
                w1ht = pw.tile([P, KH, H], F8)
                ecT16 = pw.tile([P, KH, NBS], F16)
                encwc16 = pw.tile([P, 2, 3 * H], F8)
                gixt = pw.tile([P, KG, TS, BC], F16)
                hallT = pw.tile([P, KH, T, BC], F32)
                # two separate aw tiles: a single tile would create a
                # tile-granular WAR between half-1's stt write and the
                # half-0 e-matmul reads, serializing the attention pipeline
                awA = pw.tile([P, KH // 2, NBS], F16)
                awB = pw.tile([P, KH // 2, NBS], F16)
                w2t16 = pw.tile([P, KH], F16)
                b1t = pw.tile([P, KH], F32)
                bhnrep = pw.tile([P, KH, BC], F32)
                ones1 = pw.tile([1, 1], F16)
                bd1 = pw.tile([P, BC], F16)
                bd2 = pw.tile([P, BC], F16)

                nc.sync.dma_start(out=w2t16[:], in_=w2t16_d[:])
                nc.sync.dma_start(out=bout16[:], in_=bout16_d[:])
                nc.sync.dma_start(out=b1t[:], in_=b1t_d[:])
                nc.sync.dma_start(
                    out=bhnrep[:],
                    in_=bhnrep_d[:].rearrange("p (k b) -> p k b", b=BC))
                nc.sync.dma_start(
                    out=hallT[:, :, 0, :],
                    in_=h0t_d[:].rearrange("p (k b) -> p k b", b=BC))
                nc.vector.memset(ones1[:], 1.0)
                nc.vector.memset(bd1[:], 0.0)
                nc.vector.memset(bd2[:], 0.0)
                nc.vector.memset(awA[:], 0.0)
                nc.vector.memset(awB[:], 0.0)

                # ---------------- phase 0 ----------------
                with (
                    tc.tile_pool(name="p0b", bufs=1) as p0b,
                    tc.tile_pool(name="p0bs", bufs=2) as p0bs,
                ):
                    enct16 = p0b.tile([P, KH, NBS], F16)
                    nc.sync.dma_start(out=enct16[:], in_=enct16_d[:])

                    # EcT (k-outer, stream W1e per k; 8 live psum banks)
                    with tc.tile_pool(name="ps_ec_pool", bufs=1,
                                      space="PSUM") as psec:
                        ps_ec = [psec.tile([P, NBS], F32, name=f"ps_ec{mo}")
                                 for mo in range(KH)]
                        for k in range(KH):
                            w1ek = p0bs.tile([P, H], F8, name="w1ek", tag="w1ek")
                            nc.sync.dma_start(out=w1ek[:], in_=w1et16_d[:, k, :])
                            for mo in range(KH):
                                nc.tensor.matmul(
                                    ps_ec[mo][:], w1ek[:, mo * P:(mo + 1) * P],
                                    enct16[:, k, :],
                                    start=(k == 0), stop=(k == KH - 1))
                        for mo in range(KH):
                            nc.vector.scalar_tensor_tensor(
                                ecT16[:, mo, :], ps_ec[mo][:], WSI,
                                b1t[:, mo:mo + 1].broadcast_to([P, NBS]),
                                op0=ALU.mult, op1=ALU.add)

                    # EncWc (n-chunked 256 wide, stream WcT on gpsimd ring)
                    with tc.tile_pool(name="ps_ew_pool", bufs=2,
                                      space="PSUM") as psew:
                        for n in range(NWC):
                            wcs = p0bs.tile([P, KH, 256], F8, name="wcs",
                                            tag="wcs", bufs=4)
                            nc.gpsimd.dma_start(
                                out=wcs[:],
                                in_=wct16_d[:, :, n * 256:(n + 1) * 256])
                            for mt in range(2):
                                ps_ew = psew.tile([P, 256], F32, name="ps_ew",
                                                  tag="ps_ew")
                                for k in range(KH):
                                    nc.tensor.matmul(
                                        ps_ew[:],
                                        enct16[:, k, mt * P:(mt + 1) * P],
                                        wcs[:, k, :],
                                        start=(k == 0), stop=(k == KH - 1))
                                nc.vector.tensor_scalar(
                                    encwc16[:, mt, n * 256:(n + 1) * 256],
                                    ps_ew[:], WSI, None, op0=ALU.mult)

                # GIX (input projection for all steps; needed at t=1 gates)
                with (
                    tc.tile_pool(name="p0a", bufs=1) as p0a,
                    tc.tile_pool(name="p0as", bufs=2) as p0as,
                    tc.tile_pool(name="ps_gx_pool", bufs=1, space="PSUM") as psgx,
                ):
                    xat16 = p0a.tile([P, 5, P], F16)
                    nc.scalar.dma_start(out=xat16[:], in_=xat16_d[:])
                    wxb16 = p0a.tile([P, 3 * H], F16)
                    nc.scalar.dma_start(out=wxb16[:], in_=wxb16_d[:])
                    ps_gx = [psgx.tile([P, 4, P], F32, name=f"ps_gx{g}")
                             for g in range(6)]
                    for k in range(4):
                        wxk = p0as.tile([P, 3 * H], F8, name="wxk", tag="wxk")
                        nc.scalar.dma_start(out=wxk[:], in_=wxat8_d[:, k, :])
                        for mo in range(KG):
                            nc.tensor.matmul(
                                ps_gx[mo // 4][:, mo % 4, :],
                                wxk[:, mo * P:(mo + 1) * P],
                                xat16[:, k, :], start=(k == 0), stop=False)
                    # step-1-critical recurrence weights follow on the
                    # scalar ring (arrive ~when phase 1 starts)
                    nc.scalar.dma_start(out=w1ht[:], in_=w1ht_d[:])
                    nc.scalar.dma_start(out=whht[:], in_=whht_d[:])
                    for mo in range(KG):
                        nc.tensor.matmul(
                            ps_gx[mo // 4][:, mo % 4, :],
                            wxb16[:, mo * P:(mo + 1) * P],
                            xat16[:, 4, :], start=False, stop=True)
                    for mo in range(KG):
                        nc.scalar.activation(
                            gixt[:, mo, :, :],
                            ps_gx[mo // 4][:, mo % 4, 0:TS * BC].rearrange(
                                "p (t b) -> p t b", b=BC),
                            AF.Copy, scale=WSI)

                # W_out is 8MB and only needed from t>=7; a real WAW dep on
                # a 1-element pre-write (which reads gixt) keeps the
                # scheduler from hoisting it ahead of the phase-0 streams
                woscr = pw.tile([1, 1], F16)
                nc.vector.tensor_copy(woscr[:], gixt[0:1, 0, 0, 0:1])
                nc.vector.tensor_copy(wo_all[0:1, 0, 0:1], woscr[:])
                nc.gpsimd.dma_start(out=wo_all[:], in_=woutt16_d[:])

                # filler psum pool opens once phase-0's 8-bank pools closed
                psfl_cm = tc.tile_pool(name="psfl", bufs=2, space="PSUM")
                psfl_holder["pool"] = psfl_cm.__enter__()

                nc.vector.memset(ones16[:], 1.0)

                # ---------------- phase 1: 31 steps ----------------
                with (
                    tc.tile_pool(name="p1", bufs=2) as p1,
                    tc.tile_pool(name="ps_hp_pool", bufs=1, space="PSUM") as pshp,
                    tc.tile_pool(name="ps_gh_pool", bufs=1, space="PSUM") as psgh,
                    tc.tile_pool(name="ps_gic_pool", bufs=1, space="PSUM") as psgic,
                    tc.tile_pool(name="ps_e_pool", bufs=1, space="PSUM") as pse,
                    tc.tile_pool(name="ps_a_pool", bufs=1, space="PSUM") as psa,
                ):
                    h16 = p1.tile([P, KH, BC], F16, name="h16", tag="h16")
                    nc.vector.tensor_copy(h16[:], hallT[:, :, 0, :])

                    for t in range(1, T):
                        hprev = hallT[:, :, t - 1, :]

                        if t == 3:
                            # b_out broadcast to all partitions via K=1 ones
                            # matmuls; placed here so it cannot delay the
                            # step-1-critical PE queue head
                            for n in range(NV):
                                ps_b = psfl_holder["pool"].tile(
                                    [P, NVS], F32, name="ps_b", tag="ps_o")
                                nc.tensor.matmul(
                                    ps_b[:], ones16[:],
                                    bout16[:, n * NVS:(n + 1) * NVS],
                                    start=True, stop=True)
                                nc.scalar.copy(
                                    boutrep16[:, n * NVS:(n + 1) * NVS],
                                    ps_b[:])

                        # Hproj (fp8 stationary, fp16 moving)
                        ps_hp = pshp.tile([P, KH, BC], F32, name="ps_hp",
                                          tag="hp")
                        for mo in range(KH):
                            for k in range(KH):
                                nc.tensor.matmul(
                                    ps_hp[:, mo, :],
                                    w1ht[:, k, mo * P:(mo + 1) * P],
                                    h16[:, k, :],
                                    start=(k == 0), stop=(k == KH - 1))

                        # attention: aw = tanh(EcT(+b1) + Hproj/WS), two
                        # independent tiles so the half-1 stt write cannot
                        # WAR-serialize against the half-0 e-matmul reads
                        KHH = KH // 2
                        stt_insts = []
                        for hh, aw in ((0, awA), (1, awB)):
                            ks = slice(hh * KHH, (hh + 1) * KHH)
                            stt_insts.append(nc.vector.scalar_tensor_tensor(
                                aw[:].rearrange(
                                    "p k (b s) -> p k b s", s=SP),
                                ps_hp[:, ks, :].broadcast_to(
                                    [P, KHH, BC, SP]),
                                WSI,
                                ecT16[:, ks, :].rearrange(
                                    "p k (b s) -> p k b s", s=SP),
                                op0=ALU.mult, op1=ALU.add))
                            nc.scalar.activation(aw[:], aw[:], AF.Tanh)

                        # gh (fp8 stationary, fp16 moving) -- the big one.
                        # Emitted in two halves with the e-score matmuls in
                        # between, so the scheduler slots e right after the
                        # aw tanh halves land and the softmax chain overlaps
                        # the gh tail.
                        ps_e = pse.tile([1, NBS], F32, name="ps_e", tag="e")
                        ps_gh = psgh.tile([P, KG, BC], F32, name="ps_gh",
                                          tag="gh")

                        def gh_block(mlo, mhi):
                            first = None
                            for mo in range(mlo, mhi):
                                for k in range(KH):
                                    mi = nc.tensor.matmul(
                                        ps_gh[:, mo, :],
                                        whht[:, k, mo * P:(mo + 1) * P],
                                        h16[:, k, :],
                                        start=(k == 0), stop=(k == KH - 1))
                                    if first is None:
                                        first = mi
                            return first

                        def e_block(klo, khi):
                            # ps_e accumulation group is interleaved with gh
                            # matmuls (different PSUM bank -- fine on HW)
                            last = None
                            for k in range(klo, khi):
                                aw = awA if k < KHH else awB
                                last = nc.tensor.matmul(
                                    ps_e[:], w2t16[:, k:k + 1],
                                    aw[:, k % KHH, :],
                                    start=(k == 0), stop=(k == KH - 1),
                                    skip_group_check=True)
                            return last

                        gh_block(0, 12)
                        ea = e_block(0, KHH)      # ready once tanh-A lands
                        g2 = gh_block(12, 18)
                        eb = e_block(KHH, KH)     # ready once tanh-B lands
                        g3 = gh_block(18, KG)

                        # hn only needs gh -- runs during the softmax window.
                        # The scheduler would otherwise place it BEFORE sttB
                        # in the vector stream, where its wait on the full gh
                        # block stalls the queue head and delays the whole
                        # attention chain by ~5us/step (trace-verified).
                        hn = p1.tile([P, KH, BC], F32, name="hn", tag="hn")
                        hn_inst = nc.vector.scalar_tensor_tensor(
                            hn[:], ps_gh[:, 2 * KH:KG, :], WSI, bhnrep[:],
                            op0=ALU.mult, op1=ALU.add)
                        tile.add_dep_helper(hn_inst.ins, stt_insts[1].ins,
                                            reason="hn waits on sttB")

                        # prev step's filler epilogue emitted after the
                        # attention chain so its vector adds / ACT exps
                        # cannot be scheduled ahead of sttB/tanhB
                        flush_pending()

                        # softmax: direct Exp (same ACT table set as Tanh);
                        # no max-shift (|e| small).  1/sum is folded into
                        # the K=1 transpose matmuls via their rhs scalar.
                        expe = p1.tile([1, NBS], F16, name="expe", tag="expe")
                        nc.scalar.activation(expe[:], ps_e[:], AF.Exp)
                        s4 = p1.tile([1, BC], F32, name="s4", tag="s4")
                        nc.vector.reduce_sum(
                            s4[:], expe[:].rearrange("a (b s) -> a b s", s=SP)
                            [:, :, 0:S],
                            axis=mybir.AxisListType.X)
                        r4 = p1.tile([1, BC], F16, name="r4", tag="r4")
                        with nc.allow_low_precision(
                                reason="alpha normalization was fp16 anyway"):
                            nc.vector.reciprocal(r4[:], s4[:])

                        # transpose exp(e) to partitions, normalized on the
                        # fly: ps_a[64b:64b+64, c] = expe[b-block] * r4[b]
                        ps_a = psa.tile([P, 2], F32, name="ps_a", tag="a")
                        for bb in range(BC):
                            nc.tensor.matmul(
                                ps_a[(bb % 2) * 64:(bb % 2) * 64 + 64,
                                     bb // 2:bb // 2 + 1],
                                expe[:, bb * SP:(bb + 1) * SP],
                                r4[:, bb:bb + 1], start=True, stop=True)
                        # bd1 on vector, bd2 on scalar: the two pairs of
                        # block-diag copies run in parallel
                        nc.vector.tensor_copy(bd1[0:64, 0:1], ps_a[0:64, 0:1])
                        nc.vector.tensor_copy(bd1[64:128, 1:2],
                                              ps_a[64:128, 0:1])
                        nc.scalar.copy(bd2[0:64, 2:3], ps_a[0:64, 1:2])
                        nc.scalar.copy(bd2[64:128, 3:4],
                                       ps_a[64:128, 1:2])

                        # gi_c = blockdiag(alpha) applied to EncWc (fp16)
                        ps_gic = psgic.tile([P, KG, BC], F32, name="ps_gic",
                                            tag="gic")
                        for mo in range(KG):
                            nc.tensor.matmul(
                                ps_gic[:, mo, :],
                                encwc16[:, 0, mo * P:(mo + 1) * P],
                                bd1[:], start=True, stop=False)
                            nc.tensor.matmul(
                                ps_gic[:, mo, :],
                                encwc16[:, 1, mo * P:(mo + 1) * P],
                                bd2[:], start=False, stop=True)

                        emit_filler(t)   # gate-chain window fillers
                        emit_filler(t)

                        # gates, all-tanh: sig(x) = (1+tanh(x/2))/2
                        s1 = p1.tile([P, KG, BC], F32, name="s1", tag="s1")
                        nc.vector.tensor_add(s1[:], ps_gic[:],
                                             gixt[:, :, t - 1, :])
                        s2 = p1.tile([P, 2 * KH, BC], F32, name="s2", tag="s2")
                        nc.vector.scalar_tensor_tensor(
                            s2[:], ps_gh[:, 0:2 * KH, :], WSI,
                            s1[:, 0:2 * KH, :], op0=ALU.mult, op1=ALU.add)
                        trz = p1.tile([P, 2 * KH, BC], F32, name="trz",
                                      tag="trz")
                        nc.scalar.activation(trz[:], s2[:], AF.Tanh, scale=0.5)
                        # m1 = (trz_r + 1) * hn ; s3 = s1_n + 0.5*m1
                        m1 = p1.tile([P, KH, BC], F32, name="m1", tag="m1")
                        nc.vector.scalar_tensor_tensor(
                            m1[:], trz[:, 0:KH, :], 1.0, hn[:],
                            op0=ALU.add, op1=ALU.mult)
                        s3 = p1.tile([P, KH, BC], F32, name="s3", tag="s3")
                        nc.vector.scalar_tensor_tensor(
                            s3[:], m1[:], 0.5, s1[:, 2 * KH:KG, :],
                            op0=ALU.mult, op1=ALU.add)
                        nn_t = p1.tile([P, KH, BC], F32, name="nn_t", tag="nn")
                        nc.scalar.activation(nn_t[:], s3[:], AF.Tanh)
                        # h = nn + (trz_z + 1)/2 * (hprev - nn)
                        dd = p1.tile([P, KH, BC], F32, name="dd", tag="dd")
                        nc.vector.tensor_sub(dd[:], hprev, nn_t[:])
                        m2 = p1.tile([P, KH, BC], F32, name="m2", tag="m2")
                        nc.vector.scalar_tensor_tensor(
                            m2[:], trz[:, KH:2 * KH, :], 1.0, dd[:],
                            op0=ALU.add, op1=ALU.mult)
                        # fp16 h first (unblocks next-step matmuls), f32 second
                        h16 = p1.tile([P, KH, BC], F16, name="h16", tag="h16")
                        nc.vector.scalar_tensor_tensor(
                            h16[:], m2[:], 0.5, nn_t[:],
                            op0=ALU.mult, op1=ALU.add)
                        nc.vector.scalar_tensor_tensor(
                            hallT[:, :, t, :], m2[:], 0.5, nn_t[:],
                            op0=ALU.mult, op1=ALU.add)

                        emit_filler(t)   # catch-up slot

                        # stage this step's h into the AllGather input
                        jt = (t - 1) // TCH
                        tlo, thi = _chunk(jt)
                        nc.sync.dma_start(
                            out=agin[jt][:, :].rearrange(
                                "(k p) (tr b) -> p k tr b", p=P, b=BC)
                            [:, :, t - tlo, :],
                            in_=h16[:])
                        if t == thi - 1:
                            nc.gpsimd.collective_compute(
                                "AllGather", ALU.bypass,
                                replica_groups=rg,
                                ins=[agin[jt].opt()],
                                outs=[agout[jt].opt()])
                            if t < T - 1:
                                _gather_chunk(nc, agout, hgat16, jt)

                    # chunk NCH-1 lands after the loop
                    _gather_chunk(nc, agout, hgat16, NCH - 1)

                    # leftover vocab matmuls + exp accumulation
                    while ti[0] < len(tasks):
                        emit_filler(10 ** 9)
                        flush_pending()

                    # per-core expsums out (host adds across cores + log)
                    ssum = pfl.tile([P, NCH], F32, name="ssum")
                    nc.vector.reduce_sum(ssum[:], sums[:, :, :],
                                         axis=mybir.AxisListType.X)
                    nc.sync.dma_start(out=sums_d[:], in_=ssum[:])

            psfl_cm.__exit__(None, None, None)
            pfl_cm.__exit__(None, None, None)
            pwo_cm.__exit__(None, None, None)

    nc.compile()
    return nc


def _gather_chunk(nc, agout, hgat16, j):
    """DMA the gathered fp16 h slots of chunk j straight into
    hgat16[:, :, j, :].  Row order within the chunk is (rank, t, b)."""
    tlo, thi = _chunk(j)
    w = (thi - tlo) * BC
    for k in range(KH):
        nc.gpsimd.dma_start(
            out=hgat16[:, k, j, 0:NCORES * w].rearrange(
                "p (r w) -> p r w", r=NCORES),
            in_=agout[j][:, k * P:(k + 1) * P, :].rearrange("r p w -> p r w"))


def _t8(w, nk=8):
    # [nk*128, M] -> [128, nk, M]
    m = w.shape[1]
    return np.ascontiguousarray(w.reshape(nk, P, m).transpose(1, 0, 2))


def _prep_inputs(inputs):
    enc = np.asarray(inputs["encoder_outputs"], np.float32)
    ehid = np.asarray(inputs["encoder_hidden"], np.float32)
    targets = np.asarray(inputs["targets"])
    emb = np.asarray(inputs["emb"], np.float32)
    W1 = np.asarray(inputs["attn_W1"], np.float32)
    b1 = np.asarray(inputs["attn_b1"], np.float32)
    W2 = np.asarray(inputs["attn_W2"], np.float32)
    W_ih = np.asarray(inputs["W_ih"], np.float32)
    b_ih = np.asarray(inputs["b_ih"], np.float32)
    W_hh = np.asarray(inputs["W_hh"], np.float32)
    b_hh = np.asarray(inputs["b_hh"], np.float32)
    W_out = np.asarray(inputs["W_out"], np.float32)
    b_out = np.asarray(inputs["b_out"], np.float32)

    # shared (replicated across cores); big weights in fp8 (x64 scale)
    import ml_dtypes
    f8 = ml_dtypes.float8_e4m3fn
    w1et16 = (_t8(W1[:, :H].T) * 64).astype(f8)
    w1ht = (_t8(np.ascontiguousarray(W1[:, H:]).T) * 64).astype(f8)
    wct16 = (_t8(np.ascontiguousarray(W_ih[:, Dw:]).T) * 64).astype(f8)
    whht = (_t8(W_hh.T) * 64).astype(f8)
    wxa = np.zeros((640, 3 * H), np.float32)
    wxa[:Dw] = W_ih[:, :Dw].T
    wxa[Dw] = b_ih + np.concatenate([b_hh[:2 * H], np.zeros(H, np.float32)])
    wxat8 = (_t8(wxa[:512] * 64, nk=4)).astype(f8)
    wxb16 = (wxa[512:640] * 64).astype(np.float16)
    w2t16 = np.ascontiguousarray(W2[0].reshape(KH, P).T).astype(np.float16)
    b1t = np.ascontiguousarray(b1.reshape(KH, P).T)
    bhnrep = np.ascontiguousarray(
        np.repeat(b_hh[2 * H:].reshape(KH, P).T[:, :, None], BC, axis=2)
        .reshape(P, KH * BC))

    x_all = emb[targets[:, :TS]]  # [B, TS, Dw]

    in_maps = []
    for c in range(NCORES):
        bsl = slice(c * BC, (c + 1) * BC)
        vsl = slice(c * VC, (c + 1) * VC)
        encT = np.zeros((H, BC, SP), np.float32)
        encT[:, :, :S] = enc[bsl].transpose(2, 0, 1)
        enct16 = _t8(encT.reshape(H, NBS)).astype(np.float16)
        xat = np.zeros((640, P), np.float32)
        xat[:Dw, :TS * BC] = x_all[bsl].transpose(2, 1, 0).reshape(Dw, TS * BC)
        xat[Dw, :TS * BC] = 1.0
        xat16 = _t8(xat, nk=5).astype(np.float16)
        h0t = np.ascontiguousarray(
            ehid[0, bsl].T.reshape(KH, P, BC).transpose(1, 0, 2)
            .reshape(P, KH * BC))
        woutt16 = _t8(np.ascontiguousarray(W_out[vsl]).T).astype(np.float16)
        bout16 = np.ascontiguousarray(b_out[vsl][None, :]).astype(np.float16)
        in_maps.append({
            "enct16": enct16, "w1et16": w1et16, "wct16": wct16,
            "wxat8": wxat8, "wxb16": wxb16, "xat16": xat16, "whht": whht,
            "w1ht": w1ht, "w2t16": w2t16, "b1t": b1t, "bhnrep": bhnrep,
            "h0t": h0t, "woutt16": woutt16, "bout16": bout16,
        })
    return in_maps


def kernel(**inputs):
    if "nc" not in _CACHE:
        _CACHE["nc"] = _build()
    nc = _CACHE["nc"]
    in_maps = _prep_inputs(inputs)
    res = run_bass_kernel_spmd(nc, in_maps, core_ids=list(range(NCORES)))
    # out rows per chunk j are (rank, t, b); vocab sharded on cores.
    # Final log-softmax shift happens here: logZ = log(sum_cores expsum).
    L = np.stack([res.results[c]["out"] for c in range(NCORES)])
    Ssum = np.zeros((P, NCH), np.float64)
    for c in range(NCORES):
        Ssum += res.results[c]["sums"].astype(np.float64)
    logZ = np.log(Ssum).astype(np.float32).T        # [NCH, P]
    L = L.astype(np.float32)
    out = np.empty((B, TS, V), np.float32)
    for j in range(NCH):
        tlo, thi = _chunk(j)
        nt = thi - tlo
        rows = nt * B
        seg = L[:, j, :rows, :].reshape(NCORES, NCORES, nt, BC, VC)
        seg = seg - logZ[j, :rows].reshape(1, NCORES, nt, BC, 1)
        # [vcore, rank, t, b, vc] -> [rank*BC+b, t, vcore*VC+vc]
        out[:, tlo - 1:thi - 1, :] = (
            seg.transpose(1, 3, 2, 0, 4).reshape(B, nt, V))
    return out


# revision 39
# speedup vs baseline: 1.1180x; 1.0267x over previous
"""Attention-GRU decoder (teacher forcing) on 8 TRN2 NeuronCores.

Strategy (v6):
  Phase 0 (per core, batch sharded 4 seqs/core): precompute
     EcT    = (enc @ W1_enc.T).T + b1   -- attention enc projection
     EncWc  = enc @ Wc.T (fp8)          -- context->GRU-input projection
     GIX    = x_aug @ Wx_aug            -- all-steps input projection + biases
   Inputs stream on the three DMA-capable rings (sync/scalar/gpsimd);
   the 8MB W_out DMA carries a real WAW dep on a 1-element pre-write
   that reads gixt, so the scheduler cannot hoist it ahead of the
   phase-0-critical streams.
  Phase 1: 31 sequential steps, data-parallel over batch (BC=4/core).
     h @ W1h.T and h @ W_hh.T are fp8 stationary matmuls (LDW-bound,
     N=4 moving).  Attention softmax uses direct Exp (exp+tanh share the
     exp_and_others ACT table set, so no table reloads); gate sigmoids
     stay in tanh form.  e-score matmuls are emitted before the gh block
     so the scheduler runs them as soon as the aw tanh halves land,
     hiding the softmax chain under the gh matmul stream.  h is written
     fp16-first so next-step matmuls start immediately.  Partial fp16
     AllGathers of h every 4 steps (staged per-step) gather straight
     into the (t, rank, b) hgat buffer via strided DMA; vocab-projection
     matmuls fill the PE idle windows, their outputs (+b_out) are
     written directly to the output tensor and exp-summed on the fly.
  Tail: last AllGather + leftover vocab matmuls + exp accumulation,
     one [P, NCH] expsum output.  The final log-softmax shift
     (out -= log(sum_cores expsum)) is folded into the host-side
     unshard (all O(V) reduction work stays on-chip).

kernel(**inputs) takes full inputs, returns [B, T-1, V] float32.
"""
import numpy as np

import concourse.bacc as bacc
import concourse.bass as bass
import concourse.mybir as mybir
import concourse.tile as tile
from concourse.bass_utils import run_bass_kernel_spmd

F32 = mybir.dt.float32
F16 = mybir.dt.float16
F8 = mybir.dt.float8e4
AF = mybir.ActivationFunctionType
ALU = mybir.AluOpType
WS = 64.0            # fp8 weight scale
WSI = 1.0 / WS

B, S, H, V, Dw, T = 32, 50, 1024, 32000, 512, 32
NCORES = 8
P = 128
TS = T - 1            # 31 decode steps
BC = B // NCORES      # 4 sequences per core
VC = V // NCORES      # 4000 vocab rows per core
SP = 64               # padded s-block per sequence
NBS = BC * SP         # 256 padded (b,s) columns per core
KH = H // P           # 8 hidden chunks
KG = 3 * H // P       # 24 gate chunks
NV = 8                # vocab n-chunks per core
NVS = VC // NV        # 500
TCH = 4               # steps per AllGather chunk
NCH = 8               # number of chunks (last has 3 steps)
NWC = 12              # EncWc column chunks (256 wide)

_CACHE = {}


def _chunk(j):
    tlo = TCH * j + 1
    thi = min(tlo + TCH, T)
    return tlo, thi


def _build():
    nc = bacc.Bacc("TRN2", target_bir_lowering=False, debug=False,
                   num_devices=NCORES)

    def din(name, shape, dt):
        return nc.dram_tensor(name, shape, dt, kind="ExternalInput").ap()

    enct16_d = din("enct16", [P, KH, NBS], F16)
    w1et16_d = din("w1et16", [P, KH, H], F8)
    wct16_d = din("wct16", [P, KH, 3 * H], F8)
    wxat8_d = din("wxat8", [P, 4, 3 * H], F8)
    wxb16_d = din("wxb16", [P, 3 * H], F16)
    xat16_d = din("xat16", [P, 5, P], F16)
    whht_d = din("whht", [P, KH, 3 * H], F8)
    w1ht_d = din("w1ht", [P, KH, H], F8)
    w2t16_d = din("w2t16", [P, KH], F16)
    b1t_d = din("b1t", [P, KH], F32)
    bhnrep_d = din("bhnrep", [P, KH * BC], F32)
    h0t_d = din("h0t", [P, KH * BC], F32)
    woutt16_d = din("woutt16", [P, KH, VC], F16)
    bout16_d = din("bout16", [1, VC], F16)
    out_d = nc.dram_tensor("out", [NCH, P, VC], F16,
                           kind="ExternalOutput").ap()
    sums_d = nc.dram_tensor("sums", [P, NCH], F32,
                            kind="ExternalOutput").ap()

    rg = [list(range(NCORES))]

    with tile.TileContext(nc) as tc:
        with tc.tile_pool(name="dram", bufs=1, space="DRAM") as dram:
            agin, agout = [], []
            for j in range(NCH):
                tlo, thi = _chunk(j)
                w = (thi - tlo) * BC
                agin.append(dram.tile([H, w], F16, name=f"agin{j}"))
                agout.append(dram.tile([NCORES, H, w], F16, name=f"agout{j}"))

            pwo_cm = tc.tile_pool(name="pwo", bufs=1)
            pwo = pwo_cm.__enter__()
            wo_all = pwo.tile([P, KH, VC], F16)
            hgat16 = pwo.tile([P, KH, NCH, P], F16)
            boutrep16 = pwo.tile([P, VC], F16)
            sums = pwo.tile([P, NCH, NV], F32)
            # only the last chunk's 96:128 pad rows are ever read unwritten
            nc.vector.memset(hgat16[:, :, NCH - 1, 96:P], 0.0)

            pfl_cm = tc.tile_pool(name="pfl", bufs=1)
            pfl = pfl_cm.__enter__()
            psfl_holder = {}
            ones16 = pfl.tile([1, P], F16)
            bout16 = pfl.tile([1, VC], F16)

            # ---- filler task machinery (vocab matmuls) ----
            tasks = [(j, n) for j in range(NCH) for n in range(NV)]
            ti = [0]
            pending = []

            def task_gate(j):
                if j < 6:
                    return TCH * j + 7
                if j == 6:
                    return 30
                return T + 1       # chunk 7 runs in the tail only

            def emit_filler(t):
                if ti[0] >= len(tasks):
                    return
                j, n = tasks[ti[0]]
                if t < task_gate(j):
                    return
                ti[0] += 1
                nsl = slice(n * NVS, (n + 1) * NVS)
                ps_o = psfl_holder["pool"].tile([P, NVS], F32, name="ps_o",
                                                tag="ps_o")
                for k in range(KH):
                    nc.tensor.matmul(ps_o[:], hgat16[:, k, j, :],
                                     wo_all[:, k, nsl],
                                     start=(k == 0), stop=(k == KH - 1))
                pending.append((j, n, ps_o))

            def flush_pending():
                for j, n, ps_o in pending:
                    nsl = slice(n * NVS, (n + 1) * NVS)
                    lgs = pfl.tile([P, NVS], F16, name="lgs", tag="lgs",
                                   bufs=4)
                    nc.vector.tensor_add(lgs[:], ps_o[:],
                                         boutrep16[:, nsl])
                    nc.sync.dma_start(out=out_d[j, :, nsl], in_=lgs[:])
                    etr = pfl.tile([P, NVS], F16, name="etr", tag="etr",
                                   bufs=2)
                    nc.scalar.activation(etr[:], lgs[:], AF.Exp,
                                         accum_out=sums[:, j, n:n + 1])
                pending.clear()

            with tc.tile_pool(name="pw", bufs=1) as pw:
                # ---- tiles that live through phases 0+1 ----
                whht = pw.tile([P, KH, 3 * H], F8)
                w1ht = pw.tile([P, KH, H], F8)
                ecT16 = pw.tile([P, KH, NBS], F16)
                encwc16 = pw.tile([P, 2, 3 * H], F8)
                gixt = pw.tile([P, KG, TS, BC], F16)
                hallT = pw.tile([P, KH, T, BC], F32)
                # two separate aw tiles: a single tile would create a
                # tile-granular WAR between half-1's stt write and the
                # half-0 e-matmul reads, serializing the attention pipeline
                awA = pw.tile([P, KH // 2, NBS], F16)
                awB = pw.tile([P, KH // 2, NBS], F16)
                w2t16 = pw.tile([P, KH], F16)
                b1t = pw.tile([P, KH], F32)
                bhnrep = pw.tile([P, KH, BC], F32)
                ones1 = pw.tile([1, 1], F16)
                bd1 = pw.tile([P, BC], F16)
                bd2 = pw.tile([P, BC], F16)

                nc.sync.dma_start(out=w2t16[:], in_=w2t16_d[:])
                nc.sync.dma_start(out=bout16[:], in_=bout16_d[:])
                nc.sync.dma_start(out=b1t[:], in_=b1t_d[:])
                nc.sync.dma_start(
                    out=bhnrep[:],
                    in_=bhnrep_d[:].rearrange("p (k b) -> p k b", b=BC))
                nc.sync.dma_start(
                    out=hallT[:, :, 0, :],
                    in_=h0t_d[:].rearrange("p (k b) -> p k b", b=BC))
                nc.vector.memset(ones1[:], 1.0)
                nc.vector.memset(bd1[:], 0.0)
                nc.vector.memset(bd2[:], 0.0)
                nc.vector.memset(awA[:], 0.0)
                nc.vector.memset(awB[:], 0.0)

                # ---------------- phase 0 ----------------
                with (
                    tc.tile_pool(name="p0b", bufs=1) as p0b,
                    tc.tile_pool(name="p0bs", bufs=2) as p0bs,
                ):
                    enct16 = p0b.tile([P, KH, NBS], F16)
                    nc.sync.dma_start(out=enct16[:], in_=enct16_d[:])

                    # EcT (k-outer, stream W1e per k; 8 live psum banks)
                    with tc.tile_pool(name="ps_ec_pool", bufs=1,
                                      space="PSUM") as psec:
                        ps_ec = [psec.tile([P, NBS], F32, name=f"ps_ec{mo}")
                                 for mo in range(KH)]
                        for k in range(KH):
                            w1ek = p0bs.tile([P, H], F8, name="w1ek", tag="w1ek", bufs=3)
                            nc.sync.dma_start(out=w1ek[:], in_=w1et16_d[:, k, :])
                            for mo in range(KH):
                                nc.tensor.matmul(
                                    ps_ec[mo][:], w1ek[:, mo * P:(mo + 1) * P],
                                    enct16[:, k, :],
                                    start=(k == 0), stop=(k == KH - 1))
                        for mo in range(KH):
                            nc.vector.scalar_tensor_tensor(
                                ecT16[:, mo, :], ps_ec[mo][:], WSI,
                                b1t[:, mo:mo + 1].broadcast_to([P, NBS]),
                                op0=ALU.mult, op1=ALU.add)

                    # EncWc (n-chunked 256 wide, stream WcT on gpsimd ring)
                    with tc.tile_pool(name="ps_ew_pool", bufs=2,
                                      space="PSUM") as psew:
                        for n in range(NWC):
                            wcs = p0bs.tile([P, KH, 256], F8, name="wcs",
                                            tag="wcs", bufs=6)
                            nc.gpsimd.dma_start(
                                out=wcs[:],
                                in_=wct16_d[:, :, n * 256:(n + 1) * 256])
                            for mt in range(2):
                                ps_ew = psew.tile([P, 256], F32, name="ps_ew",
                                                  tag="ps_ew")
                                for k in range(KH):
                                    nc.tensor.matmul(
                                        ps_ew[:],
                                        enct16[:, k, mt * P:(mt + 1) * P],
                                        wcs[:, k, :],
                                        start=(k == 0), stop=(k == KH - 1))
                                nc.vector.tensor_scalar(
                                    encwc16[:, mt, n * 256:(n + 1) * 256],
                                    ps_ew[:], WSI, None, op0=ALU.mult)

                # GIX (input projection for all steps; needed at t=1 gates)
                with (
                    tc.tile_pool(name="p0a", bufs=1) as p0a,
                    tc.tile_pool(name="p0as", bufs=2) as p0as,
                    tc.tile_pool(name="ps_gx_pool", bufs=1, space="PSUM") as psgx,
                ):
                    xat16 = p0a.tile([P, 5, P], F16)
                    nc.scalar.dma_start(out=xat16[:], in_=xat16_d[:])
                    wxb16 = p0a.tile([P, 3 * H], F16)
                    nc.scalar.dma_start(out=wxb16[:], in_=wxb16_d[:])
                    ps_gx = [psgx.tile([P, 4, P], F32, name=f"ps_gx{g}")
                             for g in range(6)]
                    for k in range(4):
                        wxk = p0as.tile([P, 3 * H], F8, name="wxk", tag="wxk")
                        nc.scalar.dma_start(out=wxk[:], in_=wxat8_d[:, k, :])
                        for mo in range(KG):
                            nc.tensor.matmul(
                                ps_gx[mo // 4][:, mo % 4, :],
                                wxk[:, mo * P:(mo + 1) * P],
                                xat16[:, k, :], start=(k == 0), stop=False)
                    # step-1-critical recurrence weights follow on the
                    # scalar ring (arrive ~when phase 1 starts)
                    nc.scalar.dma_start(out=w1ht[:], in_=w1ht_d[:])
                    nc.scalar.dma_start(out=whht[:], in_=whht_d[:])
                    for mo in range(KG):
                        nc.tensor.matmul(
                            ps_gx[mo // 4][:, mo % 4, :],
                            wxb16[:, mo * P:(mo + 1) * P],
                            xat16[:, 4, :], start=False, stop=True)
                    for mo in range(KG):
                        nc.scalar.activation(
                            gixt[:, mo, :, :],
                            ps_gx[mo // 4][:, mo % 4, 0:TS * BC].rearrange(
                                "p (t b) -> p t b", b=BC),
                            AF.Copy, scale=WSI)

                # W_out is 8MB and only needed from t>=7; a real WAW dep on
                # a 1-element pre-write (which reads gixt) keeps the
                # scheduler from hoisting it ahead of the phase-0 streams
                woscr = pw.tile([1, 1], F16)
                nc.vector.tensor_copy(woscr[:], gixt[0:1, 0, 0, 0:1])
                nc.vector.tensor_copy(wo_all[0:1, 0, 0:1], woscr[:])
                nc.gpsimd.dma_start(out=wo_all[:], in_=woutt16_d[:])

                # filler psum pool opens once phase-0's 8-bank pools closed
                psfl_cm = tc.tile_pool(name="psfl", bufs=2, space="PSUM")
                psfl_holder["pool"] = psfl_cm.__enter__()

                nc.vector.memset(ones16[:], 1.0)

                # ---------------- phase 1: 31 steps ----------------
                with (
                    tc.tile_pool(name="p1", bufs=2) as p1,
                    tc.tile_pool(name="ps_hp_pool", bufs=1, space="PSUM") as pshp,
                    tc.tile_pool(name="ps_gh_pool", bufs=1, space="PSUM") as psgh,
                    tc.tile_pool(name="ps_gic_pool", bufs=1, space="PSUM") as psgic,
                    tc.tile_pool(name="ps_e_pool", bufs=1, space="PSUM") as pse,
                    tc.tile_pool(name="ps_a_pool", bufs=1, space="PSUM") as psa,
                ):
                    h16 = p1.tile([P, KH, BC], F16, name="h16", tag="h16")
                    nc.vector.tensor_copy(h16[:], hallT[:, :, 0, :])

                    for t in range(1, T):
                        hprev = hallT[:, :, t - 1, :]

                        if t == 3:
                            # b_out broadcast to all partitions via K=1 ones
                            # matmuls; placed here so it cannot delay the
                            # step-1-critical PE queue head
                            for n in range(NV):
                                ps_b = psfl_holder["pool"].tile(
                                    [P, NVS], F32, name="ps_b", tag="ps_o")
                                nc.tensor.matmul(
                                    ps_b[:], ones16[:],
                                    bout16[:, n * NVS:(n + 1) * NVS],
                                    start=True, stop=True)
                                nc.scalar.copy(
                                    boutrep16[:, n * NVS:(n + 1) * NVS],
                                    ps_b[:])

                        # Hproj (fp8 stationary, fp16 moving)
                        ps_hp = pshp.tile([P, KH, BC], F32, name="ps_hp",
                                          tag="hp")
                        for mo in range(KH):
                            for k in range(KH):
                                nc.tensor.matmul(
                                    ps_hp[:, mo, :],
                                    w1ht[:, k, mo * P:(mo + 1) * P],
                                    h16[:, k, :],
                                    start=(k == 0), stop=(k == KH - 1))

                        # attention: aw = tanh(EcT(+b1) + Hproj/WS), two
                        # independent tiles so the half-1 stt write cannot
                        # WAR-serialize against the half-0 e-matmul reads
                        KHH = KH // 2
                        stt_insts = []
                        for hh, aw in ((0, awA), (1, awB)):
                            ks = slice(hh * KHH, (hh + 1) * KHH)
                            stt_insts.append(nc.vector.scalar_tensor_tensor(
                                aw[:].rearrange(
                                    "p k (b s) -> p k b s", s=SP),
                                ps_hp[:, ks, :].broadcast_to(
                                    [P, KHH, BC, SP]),
                                WSI,
                                ecT16[:, ks, :].rearrange(
                                    "p k (b s) -> p k b s", s=SP),
                                op0=ALU.mult, op1=ALU.add))
                            nc.scalar.activation(aw[:], aw[:], AF.Tanh)

                        # gh (fp8 stationary, fp16 moving) -- the big one.
                        # Emitted in two halves with the e-score matmuls in
                        # between, so the scheduler slots e right after the
                        # aw tanh halves land and the softmax chain overlaps
                        # the gh tail.
                        ps_e = pse.tile([1, NBS], F32, name="ps_e", tag="e")
                        ps_gh = psgh.tile([P, KG, BC], F32, name="ps_gh",
                                          tag="gh")

                        def gh_block(mlo, mhi):
                            first = None
                            for mo in range(mlo, mhi):
                                for k in range(KH):
                                    mi = nc.tensor.matmul(
                                        ps_gh[:, mo, :],
                                        whht[:, k, mo * P:(mo + 1) * P],
                                        h16[:, k, :],
                                        start=(k == 0), stop=(k == KH - 1))
                                    if first is None:
                                        first = mi
                            return first

                        def e_block(klo, khi):
                            # ps_e accumulation group is interleaved with gh
                            # matmuls (different PSUM bank -- fine on HW)
                            last = None
                            for k in range(klo, khi):
                                aw = awA if k < KHH else awB
                                last = nc.tensor.matmul(
                                    ps_e[:], w2t16[:, k:k + 1],
                                    aw[:, k % KHH, :],
                                    start=(k == 0), stop=(k == KH - 1),
                                    skip_group_check=True)
                            return last

                        gh_block(0, 12)
                        ea = e_block(0, KHH)      # ready once tanh-A lands
                        g2 = gh_block(12, 18)
                        eb = e_block(KHH, KH)     # ready once tanh-B lands
                        g3 = gh_block(18, KG)

                        # hn only needs gh -- runs during the softmax window.
                        # The scheduler would otherwise place it BEFORE sttB
                        # in the vector stream, where its wait on the full gh
                        # block stalls the queue head and delays the whole
                        # attention chain by ~5us/step (trace-verified).
                        hn = p1.tile([P, KH, BC], F32, name="hn", tag="hn")
                        hn_inst = nc.vector.scalar_tensor_tensor(
                            hn[:], ps_gh[:, 2 * KH:KG, :], WSI, bhnrep[:],
                            op0=ALU.mult, op1=ALU.add)
                        tile.add_dep_helper(hn_inst.ins, stt_insts[1].ins,
                                            reason="hn waits on sttB")

                        # prev step's filler epilogue emitted after the
                        # attention chain so its vector adds / ACT exps
                        # cannot be scheduled ahead of sttB/tanhB
                        flush_pending()

                        # softmax: direct Exp (same ACT table set as Tanh);
                        # no max-shift (|e| small).  1/sum is folded into
                        # the K=1 transpose matmuls via their rhs scalar.
                        expe = p1.tile([1, NBS], F16, name="expe", tag="expe")
                        nc.scalar.activation(expe[:], ps_e[:], AF.Exp)
                        s4 = p1.tile([1, BC], F32, name="s4", tag="s4")
                        nc.vector.reduce_sum(
                            s4[:], expe[:].rearrange("a (b s) -> a b s", s=SP)
                            [:, :, 0:S],
                            axis=mybir.AxisListType.X)
                        r4 = p1.tile([1, BC], F16, name="r4", tag="r4")
                        with nc.allow_low_precision(
                                reason="alpha normalization was fp16 anyway"):
                            nc.vector.reciprocal(r4[:], s4[:])

                        # transpose exp(e) to partitions, normalized on the
                        # fly: ps_a[64b:64b+64, c] = expe[b-block] * r4[b]
                        ps_a = psa.tile([P, 2], F32, name="ps_a", tag="a")
                        for bb in range(BC):
                            nc.tensor.matmul(
                                ps_a[(bb % 2) * 64:(bb % 2) * 64 + 64,
                                     bb // 2:bb // 2 + 1],
                                expe[:, bb * SP:(bb + 1) * SP],
                                r4[:, bb:bb + 1], start=True, stop=True)
                        # bd1 on vector, bd2 on scalar: the two pairs of
                        # block-diag copies run in parallel
                        nc.vector.tensor_copy(bd1[0:64, 0:1], ps_a[0:64, 0:1])
                        nc.vector.tensor_copy(bd1[64:128, 1:2],
                                              ps_a[64:128, 0:1])
                        nc.scalar.copy(bd2[0:64, 2:3], ps_a[0:64, 1:2])
                        nc.scalar.copy(bd2[64:128, 3:4],
                                       ps_a[64:128, 1:2])

                        # gi_c = blockdiag(alpha) applied to EncWc (fp16)
                        ps_gic = psgic.tile([P, KG, BC], F32, name="ps_gic",
                                            tag="gic")
                        for mo in range(KG):
                            nc.tensor.matmul(
                                ps_gic[:, mo, :],
                                encwc16[:, 0, mo * P:(mo + 1) * P],
                                bd1[:], start=True, stop=False)
                            nc.tensor.matmul(
                                ps_gic[:, mo, :],
                                encwc16[:, 1, mo * P:(mo + 1) * P],
                                bd2[:], start=False, stop=True)

                        emit_filler(t)   # gate-chain window fillers
                        emit_filler(t)

                        # gates, all-tanh: sig(x) = (1+tanh(x/2))/2
                        s1 = p1.tile([P, KG, BC], F32, name="s1", tag="s1")
                        nc.vector.tensor_add(s1[:], ps_gic[:],
                                             gixt[:, :, t - 1, :])
                        s2 = p1.tile([P, 2 * KH, BC], F32, name="s2", tag="s2")
                        nc.vector.scalar_tensor_tensor(
                            s2[:], ps_gh[:, 0:2 * KH, :], WSI,
                            s1[:, 0:2 * KH, :], op0=ALU.mult, op1=ALU.add)
                        trz = p1.tile([P, 2 * KH, BC], F32, name="trz",
                                      tag="trz")
                        nc.scalar.activation(trz[:], s2[:], AF.Tanh, scale=0.5)
                        # m1 = (trz_r + 1) * hn ; s3 = s1_n + 0.5*m1
                        m1 = p1.tile([P, KH, BC], F32, name="m1", tag="m1")
                        nc.vector.scalar_tensor_tensor(
                            m1[:], trz[:, 0:KH, :], 1.0, hn[:],
                            op0=ALU.add, op1=ALU.mult)
                        s3 = p1.tile([P, KH, BC], F32, name="s3", tag="s3")
                        nc.vector.scalar_tensor_tensor(
                            s3[:], m1[:], 0.5, s1[:, 2 * KH:KG, :],
                            op0=ALU.mult, op1=ALU.add)
                        nn_t = p1.tile([P, KH, BC], F32, name="nn_t", tag="nn")
                        nc.scalar.activation(nn_t[:], s3[:], AF.Tanh)
                        # h = nn + (trz_z + 1)/2 * (hprev - nn)
                        dd = p1.tile([P, KH, BC], F32, name="dd", tag="dd")
                        nc.vector.tensor_sub(dd[:], hprev, nn_t[:])
                        m2 = p1.tile([P, KH, BC], F32, name="m2", tag="m2")
                        nc.vector.scalar_tensor_tensor(
                            m2[:], trz[:, KH:2 * KH, :], 1.0, dd[:],
                            op0=ALU.add, op1=ALU.mult)
                        # fp16 h first (unblocks next-step matmuls), f32 second
                        h16 = p1.tile([P, KH, BC], F16, name="h16", tag="h16")
                        nc.vector.scalar_tensor_tensor(
                            h16[:], m2[:], 0.5, nn_t[:],
                            op0=ALU.mult, op1=ALU.add)
                        nc.vector.scalar_tensor_tensor(
                            hallT[:, :, t, :], m2[:], 0.5, nn_t[:],
                            op0=ALU.mult, op1=ALU.add)

                        emit_filler(t)   # catch-up slot

                        # stage this step's h into the AllGather input
                        jt = (t - 1) // TCH
                        tlo, thi = _chunk(jt)
                        nc.sync.dma_start(
                            out=agin[jt][:, :].rearrange(
                                "(k p) (tr b) -> p k tr b", p=P, b=BC)
                            [:, :, t - tlo, :],
                            in_=h16[:])
                        if t == thi - 1:
                            nc.gpsimd.collective_compute(
                                "AllGather", ALU.bypass,
                                replica_groups=rg,
                                ins=[agin[jt].opt()],
                                outs=[agout[jt].opt()])
                            if t < T - 1:
                                _gather_chunk(nc, agout, hgat16, jt)

                    # chunk NCH-1 lands after the loop
                    _gather_chunk(nc, agout, hgat16, NCH - 1)

                    # leftover vocab matmuls + exp accumulation
                    while ti[0] < len(tasks):
                        emit_filler(10 ** 9)
                        flush_pending()

                    # per-core expsums out (host adds across cores + log)
                    ssum = pfl.tile([P, NCH], F32, name="ssum")
                    nc.vector.reduce_sum(ssum[:], sums[:, :, :],
                                         axis=mybir.AxisListType.X)
                    nc.sync.dma_start(out=sums_d[:], in_=ssum[:])

            psfl_cm.__exit__(None, None, None)
            pfl_cm.__exit__(None, None, None)
            pwo_cm.__exit__(None, None, None)

    nc.compile()
    return nc


def _gather_chunk(nc, agout, hgat16, j):
    """DMA the gathered fp16 h slots of chunk j straight into
    hgat16[:, :, j, :].  Row order within the chunk is (rank, t, b)."""
    tlo, thi = _chunk(j)
    w = (thi - tlo) * BC
    for k in range(KH):
        nc.gpsimd.dma_start(
            out=hgat16[:, k, j, 0:NCORES * w].rearrange(
                "p (r w) -> p r w", r=NCORES),
            in_=agout[j][:, k * P:(k + 1) * P, :].rearrange("r p w -> p r w"))


def _t8(w, nk=8):
    # [nk*128, M] -> [128, nk, M]
    m = w.shape[1]
    return np.ascontiguousarray(w.reshape(nk, P, m).transpose(1, 0, 2))


def _prep_inputs(inputs):
    enc = np.asarray(inputs["encoder_outputs"], np.float32)
    ehid = np.asarray(inputs["encoder_hidden"], np.float32)
    targets = np.asarray(inputs["targets"])
    emb = np.asarray(inputs["emb"], np.float32)
    W1 = np.asarray(inputs["attn_W1"], np.float32)
    b1 = np.asarray(inputs["attn_b1"], np.float32)
    W2 = np.asarray(inputs["attn_W2"], np.float32)
    W_ih = np.asarray(inputs["W_ih"], np.float32)
    b_ih = np.asarray(inputs["b_ih"], np.float32)
    W_hh = np.asarray(inputs["W_hh"], np.float32)
    b_hh = np.asarray(inputs["b_hh"], np.float32)
    W_out = np.asarray(inputs["W_out"], np.float32)
    b_out = np.asarray(inputs["b_out"], np.float32)

    # shared (replicated across cores); big weights in fp8 (x64 scale)
    import ml_dtypes
    f8 = ml_dtypes.float8_e4m3fn
    w1et16 = (_t8(W1[:, :H].T) * 64).astype(f8)
    w1ht = (_t8(np.ascontiguousarray(W1[:, H:]).T) * 64).astype(f8)
    wct16 = (_t8(np.ascontiguousarray(W_ih[:, Dw:]).T) * 64).astype(f8)
    whht = (_t8(W_hh.T) * 64).astype(f8)
    wxa = np.zeros((640, 3 * H), np.float32)
    wxa[:Dw] = W_ih[:, :Dw].T
    wxa[Dw] = b_ih + np.concatenate([b_hh[:2 * H], np.zeros(H, np.float32)])
    wxat8 = (_t8(wxa[:512] * 64, nk=4)).astype(f8)
    wxb16 = (wxa[512:640] * 64).astype(np.float16)
    w2t16 = np.ascontiguousarray(W2[0].reshape(KH, P).T).astype(np.float16)
    b1t = np.ascontiguousarray(b1.reshape(KH, P).T)
    bhnrep = np.ascontiguousarray(
        np.repeat(b_hh[2 * H:].reshape(KH, P).T[:, :, None], BC, axis=2)
        .reshape(P, KH * BC))

    x_all = emb[targets[:, :TS]]  # [B, TS, Dw]

    in_maps = []
    for c in range(NCORES):
        bsl = slice(c * BC, (c + 1) * BC)
        vsl = slice(c * VC, (c + 1) * VC)
        encT = np.zeros((H, BC, SP), np.float32)
        encT[:, :, :S] = enc[bsl].transpose(2, 0, 1)
        enct16 = _t8(encT.reshape(H, NBS)).astype(np.float16)
        xat = np.zeros((640, P), np.float32)
        xat[:Dw, :TS * BC] = x_all[bsl].transpose(2, 1, 0).reshape(Dw, TS * BC)
        xat[Dw, :TS * BC] = 1.0
        xat16 = _t8(xat, nk=5).astype(np.float16)
        h0t = np.ascontiguousarray(
            ehid[0, bsl].T.reshape(KH, P, BC).transpose(1, 0, 2)
            .reshape(P, KH * BC))
        woutt16 = _t8(np.ascontiguousarray(W_out[vsl]).T).astype(np.float16)
        bout16 = np.ascontiguousarray(b_out[vsl][None, :]).astype(np.float16)
        in_maps.append({
            "enct16": enct16, "w1et16": w1et16, "wct16": wct16,
            "wxat8": wxat8, "wxb16": wxb16, "xat16": xat16, "whht": whht,
            "w1ht": w1ht, "w2t16": w2t16, "b1t": b1t, "bhnrep": bhnrep,
            "h0t": h0t, "woutt16": woutt16, "bout16": bout16,
        })
    return in_maps


def kernel(**inputs):
    if "nc" not in _CACHE:
        _CACHE["nc"] = _build()
    nc = _CACHE["nc"]
    in_maps = _prep_inputs(inputs)
    res = run_bass_kernel_spmd(nc, in_maps, core_ids=list(range(NCORES)))
    # out rows per chunk j are (rank, t, b); vocab sharded on cores.
    # Final log-softmax shift happens here: logZ = log(sum_cores expsum).
    L = np.stack([res.results[c]["out"] for c in range(NCORES)])
    Ssum = np.zeros((P, NCH), np.float64)
    for c in range(NCORES):
        Ssum += res.results[c]["sums"].astype(np.float64)
    logZ = np.log(Ssum).astype(np.float32).T        # [NCH, P]
    L = L.astype(np.float32)
    out = np.empty((B, TS, V), np.float32)
    for j in range(NCH):
        tlo, thi = _chunk(j)
        nt = thi - tlo
        rows = nt * B
        seg = L[:, j, :rows, :].reshape(NCORES, NCORES, nt, BC, VC)
        seg = seg - logZ[j, :rows].reshape(1, NCORES, nt, BC, 1)
        # [vcore, rank, t, b, vc] -> [rank*BC+b, t, vcore*VC+vc]
        out[:, tlo - 1:thi - 1, :] = (
            seg.transpose(1, 3, 2, 0, 4).reshape(B, nt, V))
    return out
